# revision 25
# baseline (speedup 1.0000x reference)
"""Trainium2 Bass kernel: row-parallel linear  y = einsum('sbk,nk->sbn', x, W) + bias.

Strategy
--------
Full inputs arrive on the host. We flatten (seq, batch) -> M = 8192 rows and
shard M across the 8 NeuronCores (1024 rows each); every core streams the full
weight and computes its [1024, 4096] slice of the output.

The correctness gate is rel_err < 2e-2 (max-abs over max-abs), which a single
reduced-precision GEMM pass meets comfortably:
  - "fp8*" (default fp8e): one e4m3 DoubleRow pass, measured 1.361e-2 rel err,
    ~0.97 ms/GEMM — the PE issue-rate ceiling for DoubleRow (256-deep
    contraction per matmul at ~241 ns for a [256]x[128]x[512] instruction).
  - "bf1": one bf16 pass, measured 8.4e-4 rel err, ~1.61 ms/GEMM (safe
    fallback, PE-bound at ~197 ns per [128]x[128]x[512] matmul).
Host-side quantization makes the device error deterministic: products are
exact in fp8/bf16 and accumulate in fp32 PSUM, so the harness re-measures
the same 1.361e-2 bit-for-bit.

Perf notes (measured on these cores):
  - per-dma_start fixed cost is ~1.5-2 us and DMA count, not bytes, dominated
    the old 3-pass baseline (~2100 DMAs -> 4.8-6.7 ms). Batching W into
    [P, 16, n_chunk] tiles (112 DMAs total) keeps the stream fully hidden.
  - W-pool depth matters: 6 tiles of prefetch absorb tunnel/HBM jitter.
  - All-core wall-clock through PJRT has +-1.5 ms per-call jitter; _bench
    times 3 back-to-back calls per attempt with a single sync and reports
    the min over interleaved (reps=1, reps=N) pair deltas.

Device layout: operands are staged in DRAM with the contraction dim on the
partition axis: x as [n_mb, P, ko, m_block] and W as [P, ko, n] with
k = ko*128 + p, so every SBUF tile load is contiguous-per-partition.

Per core: loop over m-blocks; per m-block the full-K x strip stays resident
in SBUF (loaded as ko-chunked tiles so matmuls start as soon as their chunk
lands and the next block prefetches into spare pool slots); W streams through
once per m-block; 8 PSUM banks hold the (m-strip x n-tile) accumulators
across the whole K loop, evicted once per n-chunk via VectorE.
"""

import os

import numpy as np
import ml_dtypes

BF16 = ml_dtypes.bfloat16
E4M3 = ml_dtypes.float8_e4m3  # TRN semantics: max normal +-240

# Problem shapes (hardcoded per contest contract).
SEQ, BATCH, D_FF, D_MODEL = 2048, 4, 16384, 4096
N_CORES = 8
P = 128

M_FULL = SEQ * BATCH            # 8192
M_CORE = M_FULL // N_CORES      # 1024

MM_N = 512                      # matmul free dim (one fp32 PSUM bank)
KO_LD = 16                      # ko chunks per x load tile

W_SCALE = 128.0                 # fp8: weight pre-scale (power of two, exact)

# Exec-time of the last hardware benchmark (ns), populated when KERNEL_BENCH>0.
LAST_EXEC_NS = None
LAST_RESULTS = None

_BUILD_CACHE = {}
_RUNNER_CACHE = {}


def _build_bf1(k, m_core, n, m_block=512, n_chunk=1024, w_ld=None, reps=1):
    """Single-pass bf16 GEMM: out[m_core, n] = x[m_core, k] @ w[n, k]^T.

    PSUM holds (m_block/128) x (n_chunk/512) fp32 accumulators across the
    full K loop; consecutive matmuls rotate banks. W is streamed once per
    m-block in [P, w_ld, n_chunk] tiles (per-dma_start fixed cost ~1.5us
    dominates below ~1MB transfers, so batch ko planes per DMA); x tiles
    are ko-chunked for fine-grained deps + prefetch; evictions are paired
    into one 512KB output DMA per psum pair."""
    import concourse.mybir as mybir
    import concourse.tile as tile
    from concourse import bacc

    if w_ld is None:
        w_ld = int(os.environ.get("KERNEL_WLD", "4"))
    ko_n = k // P
    n_mb = m_core // m_block
    n_nc = n // n_chunk
    ms_n = m_block // P
    nt_n = n_chunk // MM_N
    n_ld = ko_n // KO_LD
    assert ms_n * nt_n <= 8, "PSUM banks exceeded"
    assert KO_LD % w_ld == 0

    nc = bacc.Bacc(None, target_bir_lowering=False, debug=False)
    xb = nc.declare_dram_parameter("xb", [n_mb, P, ko_n, m_block],
                                   mybir.dt.bfloat16, isOutput=False)
    wb = nc.declare_dram_parameter("wb", [P, ko_n, n],
                                   mybir.dt.bfloat16, isOutput=False)
    out = nc.declare_dram_parameter("out", [m_core, n], mybir.dt.float32,
                                    isOutput=True)

    f32 = mybir.dt.float32
    bf16 = mybir.dt.bfloat16

    with tile.TileContext(nc) as tc:
        with (
            tc.tile_pool(name="xpool", bufs=n_ld + 1) as xpool,
            tc.tile_pool(name="wpool", bufs=4) as wpool,
            tc.tile_pool(name="opool", bufs=2) as opool,
            tc.tile_pool(name="pspool", bufs=8, space="PSUM") as pspool,
        ):
            for rep, mb in ((r_, m_) for r_ in range(reps) for m_ in range(n_mb)):
                xts = []
                for i in range(n_ld):
                    xt = xpool.tile([P, KO_LD, m_block], bf16, tag="xt",
                                    name=f"x_{rep}_{mb}_{i}")
                    nc.sync.dma_start(xt, xb[mb, :, i * KO_LD:(i + 1) * KO_LD, :])
                    xts.append(xt)
                m0 = mb * m_block
                for nc0 in range(n_nc):
                    c0 = nc0 * n_chunk
                    psums = [
                        pspool.tile([P, MM_N], f32, tag="ps",
                                    name=f"ps_{rep}_{mb}_{nc0}_{i}")
                        for i in range(ms_n * nt_n)
                    ]
                    for kw in range(ko_n // w_ld):
                        wt = wpool.tile([P, w_ld, n_chunk], bf16, tag="wt")
                        nc.sync.dma_start(
                            wt, wb[:, kw * w_ld:(kw + 1) * w_ld, c0:c0 + n_chunk])
                        for kj in range(w_ld):
                            ko = kw * w_ld + kj
                            first = ko == 0
                            last = ko == ko_n - 1
                            xt = xts[ko // KO_LD]
                            for ms in range(ms_n):
                                lhs = xt[:, ko % KO_LD, ms * P:(ms + 1) * P]
                                for nt in range(nt_n):
                                    nc.tensor.matmul(
                                        psums[ms * nt_n + nt],
                                        lhs,
                                        wt[:, kj, nt * MM_N:(nt + 1) * MM_N],
                                        start=first,
                                        stop=last,
                                    )
                    for ms in range(ms_n):
                        st = opool.tile([P, nt_n * MM_N], f32, tag="st")
                        for nt in range(nt_n):
                            nc.vector.tensor_copy(
                                out=st[:, nt * MM_N:(nt + 1) * MM_N],
                                in_=psums[ms * nt_n + nt])
                        nc.sync.dma_start(
                            out[m0 + ms * P:m0 + (ms + 1) * P,
                                c0:c0 + nt_n * MM_N],
                            st,
                        )
    nc.compile()
    return nc


def _build_fp8(k, m_core, n, m_block=256, n_chunk=2048, w_ld=None, reps=1,
               merge_out=False, w_bufs=4):
    """Single-pass e4m3 GEMM with DoubleRow: each matmul contracts 256 rows
    (2 ko chunks packed per PE cell). Both operands carry a [P, 2, free] AP.
    W arrives pre-scaled by W_SCALE; the host descales the output. W is
    streamed in [P, w_ld, n_chunk] tiles to amortize per-DMA fixed cost."""
    import concourse.mybir as mybir
    import concourse.tile as tile
    from concourse import bacc

    if w_ld is None:
        w_ld = int(os.environ.get("KERNEL_WLD", "4"))
    ko_n = k // P
    n_mb = m_core // m_block
    n_nc = n // n_chunk
    ms_n = m_block // P
    nt_n = n_chunk // MM_N
    n_ld = ko_n // KO_LD
    assert ms_n * nt_n <= 8, "PSUM banks exceeded"
    assert w_ld % 2 == 0 and (KO_LD % w_ld == 0 or w_ld % KO_LD == 0)

    nc = bacc.Bacc(None, target_bir_lowering=False, debug=False)
    xb = nc.declare_dram_parameter("xb", [n_mb, P, ko_n, m_block],
                                   mybir.dt.float8e4, isOutput=False)
    wb = nc.declare_dram_parameter("wb", [P, ko_n, n],
                                   mybir.dt.float8e4, isOutput=False)
    if merge_out:
        # partition-major: out[p, mg, nn] = y[mg*P + p, nn]
        out = nc.declare_dram_parameter("out", [P, m_core // P, n],
                                        mybir.dt.float32, isOutput=True)
    else:
        out = nc.declare_dram_parameter("out", [m_core, n], mybir.dt.float32,
                                        isOutput=True)

    f32 = mybir.dt.float32
    fp8 = mybir.dt.float8e4
    dr = mybir.MatmulPerfMode.DoubleRow

    with tile.TileContext(nc) as tc:
        with (
            tc.tile_pool(name="xpool", bufs=n_ld + 2) as xpool,
            tc.tile_pool(name="wpool", bufs=w_bufs) as wpool,
            tc.tile_pool(name="opool", bufs=2) as opool,
            tc.tile_pool(name="pspool", bufs=8, space="PSUM") as pspool,
        ):
            for rep, mb in ((r_, m_) for r_ in range(reps) for m_ in range(n_mb)):
                xts = []
                for i in range(n_ld):
                    xt = xpool.tile([P, KO_LD, m_block], fp8, tag="xt",
                                    name=f"x_{rep}_{mb}_{i}")
                    nc.sync.dma_start(xt, xb[mb, :, i * KO_LD:(i + 1) * KO_LD, :])
                    xts.append(xt)
                m0 = mb * m_block
                for nc0 in range(n_nc):
                    c0 = nc0 * n_chunk
                    psums = [
                        pspool.tile([P, MM_N], f32, tag="ps",
                                    name=f"ps_{rep}_{mb}_{nc0}_{i}")
                        for i in range(ms_n * nt_n)
                    ]
                    for kw in range(ko_n // w_ld):
                        wt = wpool.tile([P, w_ld, n_chunk], fp8, tag="wt")
                        nc.sync.dma_start(
                            wt, wb[:, kw * w_ld:(kw + 1) * w_ld, c0:c0 + n_chunk])
                        for kj in range(0, w_ld, 2):
                            ko = kw * w_ld + kj
                            first = ko == 0
                            last = ko == ko_n - 2
                            xt = xts[ko // KO_LD]
                            kx = ko % KO_LD
                            for ms in range(ms_n):
                                lhs = xt[:, kx:kx + 2, ms * P:(ms + 1) * P]
                                for nt in range(nt_n):
                                    nc.tensor.matmul(
                                        psums[ms * nt_n + nt],
                                        lhs,
                                        wt[:, kj:kj + 2,
                                           nt * MM_N:(nt + 1) * MM_N],
                                        start=first,
                                        stop=last,
                                        perf_mode=dr,
                                    )
                    if merge_out:
                        msg0 = m0 // P
                        st = opool.tile([P, ms_n, nt_n * MM_N], f32, tag="st")
                        for ms in range(ms_n):
                            for nt in range(nt_n):
                                nc.vector.tensor_copy(
                                    out=st[:, ms, nt * MM_N:(nt + 1) * MM_N],
                                    in_=psums[ms * nt_n + nt])
                        nc.sync.dma_start(
                            out[:, msg0:msg0 + ms_n, c0:c0 + nt_n * MM_N],
                            st,
                        )
                    else:
                        for ms in range(ms_n):
                            st = opool.tile([P, nt_n * MM_N], f32, tag="st")
                            for nt in range(nt_n):
                                nc.vector.tensor_copy(
                                    out=st[:, nt * MM_N:(nt + 1) * MM_N],
                                    in_=psums[ms * nt_n + nt])
                            nc.sync.dma_start(
                                out[m0 + ms * P:m0 + (ms + 1) * P,
                                    c0:c0 + nt_n * MM_N],
                                st,
                            )
    nc.compile()
    return nc


def _build_bf1_nomm(k, m_core, n, m_block=512, n_chunk=1024, reps=1):
    """Diagnostic: bf1's exact DMA stream with no matmuls (times pure DMA)."""
    import concourse.mybir as mybir
    import concourse.tile as tile
    from concourse import bacc

    ko_n = k // P
    n_mb = m_core // m_block
    n_nc = n // n_chunk
    n_ld = ko_n // KO_LD

    nc = bacc.Bacc(None, target_bir_lowering=False, debug=False)
    xb = nc.declare_dram_parameter("xb", [n_mb, P, ko_n, m_block],
                                   mybir.dt.bfloat16, isOutput=False)
    wb = nc.declare_dram_parameter("wb", [P, ko_n, n],
                                   mybir.dt.bfloat16, isOutput=False)
    out = nc.declare_dram_parameter("out", [m_core, n], mybir.dt.float32,
                                    isOutput=True)
    bf16 = mybir.dt.bfloat16
    with tile.TileContext(nc) as tc:
        with (
            tc.tile_pool(name="xpool", bufs=n_ld + 2) as xpool,
            tc.tile_pool(name="wpool", bufs=6) as wpool,
        ):
            for rep, mb in ((r_, m_) for r_ in range(reps) for m_ in range(n_mb)):
                for i in range(n_ld):
                    xt = xpool.tile([P, KO_LD, m_block], bf16, tag="xt",
                                    name=f"x_{rep}_{mb}_{i}")
                    nc.sync.dma_start(xt, xb[mb, :, i * KO_LD:(i + 1) * KO_LD, :])
                for nc0 in range(n_nc):
                    c0 = nc0 * n_chunk
                    for ko in range(ko_n):
                        wt = wpool.tile([P, n_chunk], bf16, tag="wt")
                        nc.sync.dma_start(wt, wb[:, ko, c0:c0 + n_chunk])
    nc.compile()
    return nc


def _build_bf1_nodma(k, m_core, n, m_block=512, n_chunk=1024, reps=1):
    """Diagnostic: bf1's exact matmul stream with W loaded once (times pure PE)."""
    import concourse.mybir as mybir
    import concourse.tile as tile
    from concourse import bacc

    ko_n = k // P
    n_mb = m_core // m_block
    n_nc = n // n_chunk
    ms_n = m_block // P
    nt_n = n_chunk // MM_N
    n_ld = ko_n // KO_LD

    nc = bacc.Bacc(None, target_bir_lowering=False, debug=False)
    xb = nc.declare_dram_parameter("xb", [n_mb, P, ko_n, m_block],
                                   mybir.dt.bfloat16, isOutput=False)
    wb = nc.declare_dram_parameter("wb", [P, ko_n, n],
                                   mybir.dt.bfloat16, isOutput=False)
    out = nc.declare_dram_parameter("out", [m_core, n], mybir.dt.float32,
                                    isOutput=True)
    f32 = mybir.dt.float32
    bf16 = mybir.dt.bfloat16
    with tile.TileContext(nc) as tc:
        with (
            tc.tile_pool(name="xpool", bufs=2) as xpool,
            tc.tile_pool(name="wpool", bufs=1) as wpool,
            tc.tile_pool(name="opool", bufs=4) as opool,
            tc.tile_pool(name="pspool", bufs=8, space="PSUM") as pspool,
        ):
            xt = xpool.tile([P, KO_LD, m_block], bf16, tag="xt")
            nc.sync.dma_start(xt, xb[0, :, 0:KO_LD, :])
            wt = wpool.tile([P, n_chunk], bf16, tag="wt")
            nc.sync.dma_start(wt, wb[:, 0, 0:n_chunk])
            for rep, mb in ((r_, m_) for r_ in range(reps) for m_ in range(n_mb)):
                m0 = mb * m_block
                for nc0 in range(n_nc):
                    c0 = nc0 * n_chunk
                    psums = [
                        pspool.tile([P, MM_N], f32, tag="ps",
                                    name=f"ps_{rep}_{mb}_{nc0}_{i}")
                        for i in range(ms_n * nt_n)
                    ]
                    for ko in range(ko_n):
                        first = ko == 0
                        last = ko == ko_n - 1
                        kj = ko % KO_LD
                        for ms in range(ms_n):
                            lhs = xt[:, kj, ms * P:(ms + 1) * P]
                            for nt in range(nt_n):
                                nc.tensor.matmul(
                                    psums[ms * nt_n + nt],
                                    lhs,
                                    wt[:, nt * MM_N:(nt + 1) * MM_N],
                                    start=first,
                                    stop=last,
                                )
                    for ms in range(ms_n):
                        for nt in range(nt_n):
                            st = opool.tile([P, MM_N], f32, tag="st")
                            nc.vector.tensor_copy(out=st, in_=psums[ms * nt_n + nt])
                            nc.sync.dma_start(
                                out[m0 + ms * P:m0 + (ms + 1) * P,
                                    c0 + nt * MM_N:c0 + (nt + 1) * MM_N],
                                st,
                            )
    nc.compile()
    return nc


_BUILDERS = {
    "bf1": _build_bf1,
    "fp8": _build_fp8,
    "fp8b": lambda k, m, n, **kw: _build_fp8(k, m, n, m_block=512, n_chunk=1024,
                                             w_ld=8, **kw),
    "fp8c": lambda k, m, n, **kw: _build_fp8(k, m, n, m_block=512, n_chunk=1024,
                                             w_ld=16, **kw),
    "fp8d": lambda k, m, n, **kw: _build_fp8(k, m, n, m_block=512, n_chunk=1024,
                                             w_ld=32, w_bufs=2, merge_out=True,
                                             **kw),
    "fp8e": lambda k, m, n, **kw: _build_fp8(k, m, n, m_block=512, n_chunk=1024,
                                             w_ld=16, w_bufs=6, **kw),
    "fp8f": lambda k, m, n, **kw: _build_fp8(k, m, n, m_block=512, n_chunk=1024,
                                             w_ld=16, w_bufs=7, **kw),
    "fp8g": lambda k, m, n, **kw: _build_fp8(k, m, n, m_block=256, n_chunk=2048,
                                             w_ld=16, w_bufs=4, **kw),
    "bf1_nomm": _build_bf1_nomm,
    "bf1_nodma": _build_bf1_nodma,
}

# variant -> (m_block for host x layout, operand dtype, W pre-scale,
#             out is partition-major [P, m_core//P, n])
VARIANT_CFG = {
    "bf1": (512, BF16, 1.0, False),
    "fp8": (256, E4M3, W_SCALE, False),
    "fp8b": (512, E4M3, W_SCALE, False),
    "fp8c": (512, E4M3, W_SCALE, False),
    "fp8d": (512, E4M3, W_SCALE, True),
    "fp8e": (512, E4M3, W_SCALE, False),
    "fp8f": (512, E4M3, W_SCALE, False),
    "fp8g": (256, E4M3, W_SCALE, False),
    "bf1_nomm": (512, BF16, 1.0, False),
    "bf1_nodma": (512, BF16, 1.0, False),
}


def _variant():
    return os.environ.get("KERNEL_VARIANT", "fp8e")


def _get_nc(k, m_core, n, **kw):
    variant = _variant()
    key = (variant, k, m_core, n, tuple(sorted(kw.items())))
    if key not in _BUILD_CACHE:
        _BUILD_CACHE[key] = _BUILDERS[variant](k, m_core, n, **kw)
    return _BUILD_CACHE[key]


def _to_pkm_blocks(a, m_block, dtype):
    """[rows, k] fp32 -> contiguous [n_mb, P, ko_n, m_block] in `dtype`
    (k = ko*128 + p)."""
    rows, k = a.shape
    n_mb = rows // m_block
    ko_n = k // P
    a = a.astype(dtype)
    a = a.reshape(n_mb, m_block, ko_n, P).transpose(0, 3, 2, 1)
    return np.ascontiguousarray(a)


def _w_to_pkn(w, dtype, scale=1.0):
    """[n, k] fp32 -> contiguous [P, ko_n, n] in `dtype`."""
    n, k = w.shape
    ko_n = k // P
    if scale != 1.0:
        w = w * np.float32(scale)
    w = w.astype(dtype)
    w = w.reshape(n, ko_n, P).transpose(2, 1, 0)
    return np.ascontiguousarray(w)


def _make_runner(nc):
    """Build the sharded PJRT executor for `nc` across the 8 cores.

    Mirrors concourse.bass2jax.run_bass_via_pjrt, but returns a reusable
    closure so repeated calls share one jit cache and inputs can stay
    device-resident for benchmarking.
    """
    import jax
    import concourse.mybir as mybir
    from concourse import bass2jax
    from jax.experimental.shard_map import shard_map
    from jax.sharding import Mesh, NamedSharding, PartitionSpec

    bass2jax.install_neuronx_cc_hook()

    partition_name = nc.partition_id_tensor.name if nc.partition_id_tensor else None
    assert nc.dbg_addr is None

    in_names, out_names, out_avals = [], [], []
    for alloc in nc.m.functions[0].allocations:
        if not isinstance(alloc, mybir.MemoryLocationSet):
            continue
        name = alloc.memorylocations[0].name
        if alloc.kind == "ExternalInput":
            if name != partition_name:
                in_names.append(name)
        elif alloc.kind == "ExternalOutput":
            out_names.append(name)
            out_avals.append(
                jax.core.ShapedArray(tuple(alloc.tensor_shape), mybir.dt.np(alloc.dtype))
            )
    n_params = len(in_names)
    n_outs = len(out_avals)
    all_in_names = tuple(in_names) + tuple(out_names)
    if partition_name is not None:
        all_in_names = all_in_names + (partition_name,)
    donate = tuple(range(n_params, n_params + n_outs))

    def _body(*args):
        operands = list(args)
        if partition_name is not None:
            operands.append(bass2jax.partition_id_tensor())
        outs = bass2jax._bass_exec_p.bind(
            *operands,
            out_avals=tuple(out_avals),
            in_names=all_in_names,
            out_names=tuple(out_names),
            lowering_input_output_aliases=(),
            sim_require_finite=True,
            sim_require_nnan=True,
            nc=nc,
        )
        return tuple(outs)

    devices = jax.devices()[:N_CORES]
    assert len(devices) == N_CORES
    mesh = Mesh(np.asarray(devices), ("core",))
    spec = PartitionSpec("core")
    sharded = jax.jit(
        shard_map(
            _body,
            mesh=mesh,
            in_specs=(spec,) * (n_params + n_outs),
            out_specs=(spec,) * n_outs,
            check_rep=False,
        ),
        donate_argnums=donate,
        keep_unused=True,
    )
    sharding = NamedSharding(mesh, spec)
    return {
        "sharded": sharded,
        "sharding": sharding,
        "in_names": in_names,
        "out_names": out_names,
        "out_avals": out_avals,
        "n_params": n_params,
        "n_outs": n_outs,
    }


def _get_runner(nc):
    key = id(nc)
    if key not in _RUNNER_CACHE:
        _RUNNER_CACHE[key] = _make_runner(nc)
    return _RUNNER_CACHE[key]


def _run(nc, in_maps):
    """Execute the kernel across 8 cores; returns per-core output dicts."""
    import numpy as np

    r = _get_runner(nc)
    n_cores = len(in_maps)
    concat_in = [
        np.concatenate([np.asarray(m[name]) for m in in_maps], axis=0)
        for name in r["in_names"]
    ]
    concat_zeros = [
        np.zeros((n_cores * a.shape[0], *a.shape[1:]), a.dtype) for a in r["out_avals"]
    ]
    out_arrs = r["sharded"](*concat_in, *concat_zeros)
    return [
        {
            name: np.asarray(out_arrs[i]).reshape(n_cores, *r["out_avals"][i].shape)[c]
            for i, name in enumerate(r["out_names"])
        }
        for c in range(n_cores)
    ]


def _bench(in_maps, k, m_core, n, reps):
    """Measure steady-state per-GEMM time: the kernel repeated `reps` times
    inside one program, minus the reps=1 program, divided by reps-1. Fixed
    dispatch overhead cancels in the difference. Sets LAST_EXEC_NS."""
    global LAST_EXEC_NS
    import time

    import jax
    import jax.numpy as jnp
    import numpy as np

    runners = {}
    dev_in = None
    for r_reps in (1, reps):
        nc = _get_nc(k, m_core, n, reps=r_reps)
        r = _get_runner(nc)
        runners[r_reps] = r
        if dev_in is None:
            concat_in = [
                np.concatenate([np.asarray(m[name]) for m in in_maps], axis=0)
                for name in r["in_names"]
            ]
            dev_in = [jax.device_put(a, r["sharding"]) for a in concat_in]
            jax.block_until_ready(dev_in)

    def _zeros(r):
        zs = [
            jax.jit(lambda a=a: jnp.zeros(a.shape, a.dtype),
                    out_shardings=r["sharding"])()
            for a in r["out_avals"]
        ]
        jax.block_until_ready(zs)
        return zs

    n_calls = int(os.environ.get("KERNEL_BENCH_CALLS", "3"))

    def _attempt(r_reps):
        # Time n_calls back-to-back dispatches with a single final sync:
        # per-call host/tunnel jitter amortizes across the batch.
        r = runners[r_reps]
        zsets = [_zeros(r) for _ in range(n_calls)]
        t0 = time.perf_counter()
        outs = [r["sharded"](*dev_in, *zs) for zs in zsets]
        jax.block_until_ready(outs)
        return (time.perf_counter() - t0) / n_calls

    for r_reps in (1, reps):  # compile + warmup both programs first
        _attempt(r_reps)

    # Interleaved attempt pairs: slow drift in fixed overhead is common-mode
    # within a pair, so per-pair deltas are far more stable than min-of-each.
    deltas, t1s, tns = [], [], []
    for _ in range(int(os.environ.get("KERNEL_BENCH_TRIES", "6"))):
        t1 = _attempt(1)
        tn = _attempt(reps)
        t1s.append(t1)
        tns.append(tn)
        deltas.append((tn - t1) / (reps - 1))
    per_iter = min(deltas)
    LAST_EXEC_NS = int(per_iter * 1e9)
    print(f"[bench] reps=1: {[f'{a * 1e3:.2f}' for a in t1s]}")
    print(f"[bench] reps={reps}: {[f'{a * 1e3:.2f}' for a in tns]}")
    print(f"[bench] per-GEMM deltas (ms): {[f'{d * 1e3:.3f}' for d in deltas]}")
    print(f"[bench] per-GEMM: {per_iter * 1e3:.3f} ms "
          f"(fixed+1iter: {min(t1s) * 1e3:.3f} ms)")


def kernel(input_, weight, bias):
    global LAST_RESULTS

    input_ = np.asarray(input_, dtype=np.float32)
    weight = np.asarray(weight, dtype=np.float32)
    bias = np.asarray(bias, dtype=np.float32)

    seq, batch, k = input_.shape
    n = weight.shape[0]
    m_full = seq * batch
    m_core = m_full // N_CORES

    variant = _variant()
    nc = _get_nc(k, m_core, n)

    x2 = input_.reshape(m_full, k)
    m_block, dtype, w_scale, out_pm = VARIANT_CFG[variant]
    wp = _w_to_pkn(weight, dtype, scale=w_scale)

    in_maps = []
    for c in range(N_CORES):
        xp = _to_pkm_blocks(x2[c * m_core:(c + 1) * m_core], m_block, dtype)
        in_maps.append({"xb": xp, "wb": wp})

    results = _run(nc, in_maps)
    LAST_RESULTS = results

    bench_reps = int(os.environ.get("KERNEL_BENCH", "0"))
    if bench_reps > 1:
        _bench(in_maps, k, m_core, n, bench_reps)

    per_core = [results[c]["out"] for c in range(N_CORES)]
    if out_pm:
        # device layout [P, m_core//P, n]: row mg*P + p lives at [p, mg, :]
        per_core = [o.transpose(1, 0, 2).reshape(m_core, n) for o in per_core]
    out = np.concatenate(per_core, axis=0)
    if w_scale != 1.0:
        out = out * np.float32(1.0 / w_scale)
    out = out.reshape(seq, batch, n)
    if bias.any():
        out = out + bias
    return out


# revision 31
# speedup vs baseline: 1.0795x; 1.0795x over previous
"""Trainium2 Bass kernel: row-parallel linear  y = einsum('sbk,nk->sbn', x, W) + bias.

Strategy
--------
Full inputs arrive on the host. We flatten (seq, batch) -> M = 8192 rows and
shard M across the 8 NeuronCores (1024 rows each); every core streams the full
weight and computes its [1024, 4096] slice of the output.

The correctness gate is rel_err < 2e-2 (max-abs over max-abs), which a single
reduced-precision GEMM pass meets comfortably:
  - "fp8*" (default fp8e): one e4m3 DoubleRow pass, measured 1.361e-2 rel err,
    ~0.97 ms/GEMM — the PE issue-rate ceiling for DoubleRow (256-deep
    contraction per matmul at ~241 ns for a [256]x[128]x[512] instruction).
  - "bf1": one bf16 pass, measured 8.4e-4 rel err, ~1.61 ms/GEMM (safe
    fallback, PE-bound at ~197 ns per [128]x[128]x[512] matmul).
Host-side quantization makes the device error deterministic: products are
exact in fp8/bf16 and accumulate in fp32 PSUM, so the harness re-measures
the same 1.361e-2 bit-for-bit.

Perf notes (measured on these cores):
  - per-dma_start fixed cost is ~1.5-2 us and DMA count, not bytes, dominated
    the old 3-pass baseline (~2100 DMAs -> 4.8-6.7 ms). Batching W into
    [P, 16, n_chunk] tiles (112 DMAs total) keeps the stream fully hidden.
  - W-pool depth matters: 6 tiles of prefetch absorb tunnel/HBM jitter.
  - All-core wall-clock through PJRT has +-1.5 ms per-call jitter; _bench
    times 3 back-to-back calls per attempt with a single sync and reports
    the min over interleaved (reps=1, reps=N) pair deltas.

Device layout: operands are staged in DRAM with the contraction dim on the
partition axis: x as [n_mb, P, ko, m_block] and W as [P, ko, n] with
k = ko*128 + p, so every SBUF tile load is contiguous-per-partition.

Per core: loop over m-blocks; per m-block the full-K x strip stays resident
in SBUF (loaded as ko-chunked tiles so matmuls start as soon as their chunk
lands and the next block prefetches into spare pool slots); W streams through
once per m-block; 8 PSUM banks hold the (m-strip x n-tile) accumulators
across the whole K loop, evicted once per n-chunk via VectorE.
"""

import os

import numpy as np
import ml_dtypes

BF16 = ml_dtypes.bfloat16
E4M3 = ml_dtypes.float8_e4m3  # TRN semantics: max normal +-240

# Problem shapes (hardcoded per contest contract).
SEQ, BATCH, D_FF, D_MODEL = 2048, 4, 16384, 4096
N_CORES = 8
P = 128

M_FULL = SEQ * BATCH            # 8192
M_CORE = M_FULL // N_CORES      # 1024

MM_N = 512                      # matmul free dim (one fp32 PSUM bank)
KO_LD = 16                      # ko chunks per x load tile

W_SCALE = 128.0                 # fp8: weight pre-scale (power of two, exact)

# Exec-time of the last hardware benchmark (ns), populated when KERNEL_BENCH>0.
LAST_EXEC_NS = None
LAST_RESULTS = None

_BUILD_CACHE = {}
_RUNNER_CACHE = {}


def _build_bf1(k, m_core, n, m_block=512, n_chunk=1024, w_ld=None, reps=1):
    """Single-pass bf16 GEMM: out[m_core, n] = x[m_core, k] @ w[n, k]^T.

    PSUM holds (m_block/128) x (n_chunk/512) fp32 accumulators across the
    full K loop; consecutive matmuls rotate banks. W is streamed once per
    m-block in [P, w_ld, n_chunk] tiles (per-dma_start fixed cost ~1.5us
    dominates below ~1MB transfers, so batch ko planes per DMA); x tiles
    are ko-chunked for fine-grained deps + prefetch; evictions are paired
    into one 512KB output DMA per psum pair."""
    import concourse.mybir as mybir
    import concourse.tile as tile
    from concourse import bacc

    if w_ld is None:
        w_ld = int(os.environ.get("KERNEL_WLD", "4"))
    ko_n = k // P
    n_mb = m_core // m_block
    n_nc = n // n_chunk
    ms_n = m_block // P
    nt_n = n_chunk // MM_N
    n_ld = ko_n // KO_LD
    assert ms_n * nt_n <= 8, "PSUM banks exceeded"
    assert KO_LD % w_ld == 0

    nc = bacc.Bacc(None, target_bir_lowering=False, debug=False)
    xb = nc.declare_dram_parameter("xb", [n_mb, P, ko_n, m_block],
                                   mybir.dt.bfloat16, isOutput=False)
    wb = nc.declare_dram_parameter("wb", [P, ko_n, n],
                                   mybir.dt.bfloat16, isOutput=False)
    out = nc.declare_dram_parameter("out", [m_core, n], mybir.dt.float32,
                                    isOutput=True)

    f32 = mybir.dt.float32
    bf16 = mybir.dt.bfloat16

    with tile.TileContext(nc) as tc:
        with (
            tc.tile_pool(name="xpool", bufs=n_ld + 1) as xpool,
            tc.tile_pool(name="wpool", bufs=4) as wpool,
            tc.tile_pool(name="opool", bufs=2) as opool,
            tc.tile_pool(name="pspool", bufs=8, space="PSUM") as pspool,
        ):
            for rep, mb in ((r_, m_) for r_ in range(reps) for m_ in range(n_mb)):
                xts = []
                for i in range(n_ld):
                    xt = xpool.tile([P, KO_LD, m_block], bf16, tag="xt",
                                    name=f"x_{rep}_{mb}_{i}")
                    nc.sync.dma_start(xt, xb[mb, :, i * KO_LD:(i + 1) * KO_LD, :])
                    xts.append(xt)
                m0 = mb * m_block
                for nc0 in range(n_nc):
                    c0 = nc0 * n_chunk
                    psums = [
                        pspool.tile([P, MM_N], f32, tag="ps",
                                    name=f"ps_{rep}_{mb}_{nc0}_{i}")
                        for i in range(ms_n * nt_n)
                    ]
                    for kw in range(ko_n // w_ld):
                        wt = wpool.tile([P, w_ld, n_chunk], bf16, tag="wt")
                        nc.sync.dma_start(
                            wt, wb[:, kw * w_ld:(kw + 1) * w_ld, c0:c0 + n_chunk])
                        for kj in range(w_ld):
                            ko = kw * w_ld + kj
                            first = ko == 0
                            last = ko == ko_n - 1
                            xt = xts[ko // KO_LD]
                            for ms in range(ms_n):
                                lhs = xt[:, ko % KO_LD, ms * P:(ms + 1) * P]
                                for nt in range(nt_n):
                                    nc.tensor.matmul(
                                        psums[ms * nt_n + nt],
                                        lhs,
                                        wt[:, kj, nt * MM_N:(nt + 1) * MM_N],
                                        start=first,
                                        stop=last,
                                    )
                    for ms in range(ms_n):
                        st = opool.tile([P, nt_n * MM_N], f32, tag="st")
                        for nt in range(nt_n):
                            nc.vector.tensor_copy(
                                out=st[:, nt * MM_N:(nt + 1) * MM_N],
                                in_=psums[ms * nt_n + nt])
                        nc.sync.dma_start(
                            out[m0 + ms * P:m0 + (ms + 1) * P,
                                c0:c0 + nt_n * MM_N],
                            st,
                        )
    nc.compile()
    return nc


def _build_fp8(k, m_core, n, m_block=256, n_chunk=2048, w_ld=None, reps=1,
               merge_out=False, w_bufs=4, split_rings=False, pe_only=False):
    """Single-pass e4m3 GEMM with DoubleRow: each matmul contracts 256 rows
    (2 ko chunks packed per PE cell). Both operands carry a [P, 2, free] AP.
    W arrives pre-scaled by W_SCALE; the host descales the output. W is
    streamed in [P, w_ld, n_chunk] tiles to amortize per-DMA fixed cost."""
    import concourse.mybir as mybir
    import concourse.tile as tile
    from concourse import bacc

    if w_ld is None:
        w_ld = int(os.environ.get("KERNEL_WLD", "4"))
    ko_n = k // P
    n_mb = m_core // m_block
    n_nc = n // n_chunk
    ms_n = m_block // P
    nt_n = n_chunk // MM_N
    n_ld = ko_n // KO_LD
    assert ms_n * nt_n <= 8, "PSUM banks exceeded"
    assert w_ld % 2 == 0 and (KO_LD % w_ld == 0 or w_ld % KO_LD == 0)

    nc = bacc.Bacc(None, target_bir_lowering=False, debug=False)
    xb = nc.declare_dram_parameter("xb", [n_mb, P, ko_n, m_block],
                                   mybir.dt.float8e4, isOutput=False)
    wb = nc.declare_dram_parameter("wb", [P, ko_n, n],
                                   mybir.dt.float8e4, isOutput=False)
    if merge_out:
        # partition-major: out[p, mg, nn] = y[mg*P + p, nn]
        out = nc.declare_dram_parameter("out", [P, m_core // P, n],
                                        mybir.dt.float32, isOutput=True)
    else:
        out = nc.declare_dram_parameter("out", [m_core, n], mybir.dt.float32,
                                        isOutput=True)

    f32 = mybir.dt.float32
    fp8 = mybir.dt.float8e4
    dr = mybir.MatmulPerfMode.DoubleRow

    with tile.TileContext(nc) as tc:
        with (
            tc.tile_pool(name="xpool", bufs=n_ld + 2) as xpool,
            tc.tile_pool(name="wpool", bufs=w_bufs) as wpool,
            tc.tile_pool(name="opool", bufs=2) as opool,
            tc.tile_pool(name="pspool", bufs=8, space="PSUM") as pspool,
        ):
            # x/out DMAs can ride the scalar engine's HWDGE ring so the W
            # stream owns the sync ring's FIFO end-to-end.
            aux = nc.scalar if split_rings else nc.sync
            if pe_only:  # diagnostic: single W tile reused, no streaming
                wt0 = wpool.tile([P, w_ld, n_chunk], fp8, tag="wt")
                nc.sync.dma_start(wt0, wb[:, 0:w_ld, 0:n_chunk])
            for rep, mb in ((r_, m_) for r_ in range(reps) for m_ in range(n_mb)):
                xts = []
                for i in range(n_ld):
                    xt = xpool.tile([P, KO_LD, m_block], fp8, tag="xt",
                                    name=f"x_{rep}_{mb}_{i}")
                    aux.dma_start(xt, xb[mb, :, i * KO_LD:(i + 1) * KO_LD, :])
                    xts.append(xt)
                m0 = mb * m_block
                for nc0 in range(n_nc):
                    c0 = nc0 * n_chunk
                    psums = [
                        pspool.tile([P, MM_N], f32, tag="ps",
                                    name=f"ps_{rep}_{mb}_{nc0}_{i}")
                        for i in range(ms_n * nt_n)
                    ]
                    for kw in range(ko_n // w_ld):
                        if pe_only:
                            wt = wt0
                        else:
                            wt = wpool.tile([P, w_ld, n_chunk], fp8, tag="wt")
                            nc.sync.dma_start(
                                wt, wb[:, kw * w_ld:(kw + 1) * w_ld, c0:c0 + n_chunk])
                        for kj in range(0, w_ld, 2):
                            ko = kw * w_ld + kj
                            first = ko == 0
                            last = ko == ko_n - 2
                            xt = xts[ko // KO_LD]
                            kx = ko % KO_LD
                            for ms in range(ms_n):
                                lhs = xt[:, kx:kx + 2, ms * P:(ms + 1) * P]
                                for nt in range(nt_n):
                                    nc.tensor.matmul(
                                        psums[ms * nt_n + nt],
                                        lhs,
                                        wt[:, kj:kj + 2,
                                           nt * MM_N:(nt + 1) * MM_N],
                                        start=first,
                                        stop=last,
                                        perf_mode=dr,
                                    )
                    if merge_out:
                        msg0 = m0 // P
                        st = opool.tile([P, ms_n, nt_n * MM_N], f32, tag="st")
                        for ms in range(ms_n):
                            for nt in range(nt_n):
                                nc.vector.tensor_copy(
                                    out=st[:, ms, nt * MM_N:(nt + 1) * MM_N],
                                    in_=psums[ms * nt_n + nt])
                        nc.sync.dma_start(
                            out[:, msg0:msg0 + ms_n, c0:c0 + nt_n * MM_N],
                            st,
                        )
                    else:
                        for ms in range(ms_n):
                            st = opool.tile([P, nt_n * MM_N], f32, tag="st")
                            for nt in range(nt_n):
                                nc.vector.tensor_copy(
                                    out=st[:, nt * MM_N:(nt + 1) * MM_N],
                                    in_=psums[ms * nt_n + nt])
                            aux.dma_start(
                                out[m0 + ms * P:m0 + (ms + 1) * P,
                                    c0:c0 + nt_n * MM_N],
                                st,
                            )
    nc.compile()
    return nc


def _build_bf1_nomm(k, m_core, n, m_block=512, n_chunk=1024, reps=1):
    """Diagnostic: bf1's exact DMA stream with no matmuls (times pure DMA)."""
    import concourse.mybir as mybir
    import concourse.tile as tile
    from concourse import bacc

    ko_n = k // P
    n_mb = m_core // m_block
    n_nc = n // n_chunk
    n_ld = ko_n // KO_LD

    nc = bacc.Bacc(None, target_bir_lowering=False, debug=False)
    xb = nc.declare_dram_parameter("xb", [n_mb, P, ko_n, m_block],
                                   mybir.dt.bfloat16, isOutput=False)
    wb = nc.declare_dram_parameter("wb", [P, ko_n, n],
                                   mybir.dt.bfloat16, isOutput=False)
    out = nc.declare_dram_parameter("out", [m_core, n], mybir.dt.float32,
                                    isOutput=True)
    bf16 = mybir.dt.bfloat16
    with tile.TileContext(nc) as tc:
        with (
            tc.tile_pool(name="xpool", bufs=n_ld + 2) as xpool,
            tc.tile_pool(name="wpool", bufs=6) as wpool,
        ):
            for rep, mb in ((r_, m_) for r_ in range(reps) for m_ in range(n_mb)):
                for i in range(n_ld):
                    xt = xpool.tile([P, KO_LD, m_block], bf16, tag="xt",
                                    name=f"x_{rep}_{mb}_{i}")
                    nc.sync.dma_start(xt, xb[mb, :, i * KO_LD:(i + 1) * KO_LD, :])
                for nc0 in range(n_nc):
                    c0 = nc0 * n_chunk
                    for ko in range(ko_n):
                        wt = wpool.tile([P, n_chunk], bf16, tag="wt")
                        nc.sync.dma_start(wt, wb[:, ko, c0:c0 + n_chunk])
    nc.compile()
    return nc


def _build_bf1_nodma(k, m_core, n, m_block=512, n_chunk=1024, reps=1):
    """Diagnostic: bf1's exact matmul stream with W loaded once (times pure PE)."""
    import concourse.mybir as mybir
    import concourse.tile as tile
    from concourse import bacc

    ko_n = k // P
    n_mb = m_core // m_block
    n_nc = n // n_chunk
    ms_n = m_block // P
    nt_n = n_chunk // MM_N
    n_ld = ko_n // KO_LD

    nc = bacc.Bacc(None, target_bir_lowering=False, debug=False)
    xb = nc.declare_dram_parameter("xb", [n_mb, P, ko_n, m_block],
                                   mybir.dt.bfloat16, isOutput=False)
    wb = nc.declare_dram_parameter("wb", [P, ko_n, n],
                                   mybir.dt.bfloat16, isOutput=False)
    out = nc.declare_dram_parameter("out", [m_core, n], mybir.dt.float32,
                                    isOutput=True)
    f32 = mybir.dt.float32
    bf16 = mybir.dt.bfloat16
    with tile.TileContext(nc) as tc:
        with (
            tc.tile_pool(name="xpool", bufs=2) as xpool,
            tc.tile_pool(name="wpool", bufs=1) as wpool,
            tc.tile_pool(name="opool", bufs=4) as opool,
            tc.tile_pool(name="pspool", bufs=8, space="PSUM") as pspool,
        ):
            xt = xpool.tile([P, KO_LD, m_block], bf16, tag="xt")
            nc.sync.dma_start(xt, xb[0, :, 0:KO_LD, :])
            wt = wpool.tile([P, n_chunk], bf16, tag="wt")
            nc.sync.dma_start(wt, wb[:, 0, 0:n_chunk])
            for rep, mb in ((r_, m_) for r_ in range(reps) for m_ in range(n_mb)):
                m0 = mb * m_block
                for nc0 in range(n_nc):
                    c0 = nc0 * n_chunk
                    psums = [
                        pspool.tile([P, MM_N], f32, tag="ps",
                                    name=f"ps_{rep}_{mb}_{nc0}_{i}")
                        for i in range(ms_n * nt_n)
                    ]
                    for ko in range(ko_n):
                        first = ko == 0
                        last = ko == ko_n - 1
                        kj = ko % KO_LD
                        for ms in range(ms_n):
                            lhs = xt[:, kj, ms * P:(ms + 1) * P]
                            for nt in range(nt_n):
                                nc.tensor.matmul(
                                    psums[ms * nt_n + nt],
                                    lhs,
                                    wt[:, nt * MM_N:(nt + 1) * MM_N],
                                    start=first,
                                    stop=last,
                                )
                    for ms in range(ms_n):
                        for nt in range(nt_n):
                            st = opool.tile([P, MM_N], f32, tag="st")
                            nc.vector.tensor_copy(out=st, in_=psums[ms * nt_n + nt])
                            nc.sync.dma_start(
                                out[m0 + ms * P:m0 + (ms + 1) * P,
                                    c0 + nt * MM_N:c0 + (nt + 1) * MM_N],
                                st,
                            )
    nc.compile()
    return nc


_BUILDERS = {
    "bf1": _build_bf1,
    "fp8": _build_fp8,
    "fp8b": lambda k, m, n, **kw: _build_fp8(k, m, n, m_block=512, n_chunk=1024,
                                             w_ld=8, **kw),
    "fp8c": lambda k, m, n, **kw: _build_fp8(k, m, n, m_block=512, n_chunk=1024,
                                             w_ld=16, **kw),
    "fp8d": lambda k, m, n, **kw: _build_fp8(k, m, n, m_block=512, n_chunk=1024,
                                             w_ld=32, w_bufs=2, merge_out=True,
                                             **kw),
    "fp8e": lambda k, m, n, **kw: _build_fp8(k, m, n, m_block=512, n_chunk=1024,
                                             w_ld=16, w_bufs=6, **kw),
    "fp8f": lambda k, m, n, **kw: _build_fp8(k, m, n, m_block=512, n_chunk=1024,
                                             w_ld=16, w_bufs=7, **kw),
    "fp8g": lambda k, m, n, **kw: _build_fp8(k, m, n, m_block=256, n_chunk=2048,
                                             w_ld=16, w_bufs=4, **kw),
    "fp8h": lambda k, m, n, **kw: _build_fp8(k, m, n, m_block=512, n_chunk=1024,
                                             w_ld=16, w_bufs=6, split_rings=True,
                                             **kw),
    "fp8_pe": lambda k, m, n, **kw: _build_fp8(k, m, n, m_block=512,
                                               n_chunk=1024, w_ld=16, w_bufs=6,
                                               pe_only=True, **kw),
    "bf1_nomm": _build_bf1_nomm,
    "bf1_nodma": _build_bf1_nodma,
}

# variant -> (m_block for host x layout, operand dtype, W pre-scale,
#             out is partition-major [P, m_core//P, n])
VARIANT_CFG = {
    "bf1": (512, BF16, 1.0, False),
    "fp8": (256, E4M3, W_SCALE, False),
    "fp8b": (512, E4M3, W_SCALE, False),
    "fp8c": (512, E4M3, W_SCALE, False),
    "fp8d": (512, E4M3, W_SCALE, True),
    "fp8e": (512, E4M3, W_SCALE, False),
    "fp8f": (512, E4M3, W_SCALE, False),
    "fp8g": (256, E4M3, W_SCALE, False),
    "fp8h": (512, E4M3, W_SCALE, False),
    "fp8_pe": (512, E4M3, W_SCALE, False),
    "bf1_nomm": (512, BF16, 1.0, False),
    "bf1_nodma": (512, BF16, 1.0, False),
}


def _variant():
    return os.environ.get("KERNEL_VARIANT", "fp8e")


def _get_nc(k, m_core, n, **kw):
    variant = _variant()
    key = (variant, k, m_core, n, tuple(sorted(kw.items())))
    if key not in _BUILD_CACHE:
        _BUILD_CACHE[key] = _BUILDERS[variant](k, m_core, n, **kw)
    return _BUILD_CACHE[key]


def _to_pkm_blocks(a, m_block, dtype):
    """[rows, k] fp32 -> contiguous [n_mb, P, ko_n, m_block] in `dtype`
    (k = ko*128 + p)."""
    rows, k = a.shape
    n_mb = rows // m_block
    ko_n = k // P
    a = a.astype(dtype)
    a = a.reshape(n_mb, m_block, ko_n, P).transpose(0, 3, 2, 1)
    return np.ascontiguousarray(a)


def _w_to_pkn(w, dtype, scale=1.0):
    """[n, k] fp32 -> contiguous [P, ko_n, n] in `dtype`."""
    n, k = w.shape
    ko_n = k // P
    if scale != 1.0:
        w = w * np.float32(scale)
    w = w.astype(dtype)
    w = w.reshape(n, ko_n, P).transpose(2, 1, 0)
    return np.ascontiguousarray(w)


def _make_runner(nc):
    """Build the sharded PJRT executor for `nc` across the 8 cores.

    Mirrors concourse.bass2jax.run_bass_via_pjrt, but returns a reusable
    closure so repeated calls share one jit cache and inputs can stay
    device-resident for benchmarking.
    """
    import jax
    import concourse.mybir as mybir
    from concourse import bass2jax
    from jax.experimental.shard_map import shard_map
    from jax.sharding import Mesh, NamedSharding, PartitionSpec

    bass2jax.install_neuronx_cc_hook()

    partition_name = nc.partition_id_tensor.name if nc.partition_id_tensor else None
    assert nc.dbg_addr is None

    in_names, out_names, out_avals = [], [], []
    for alloc in nc.m.functions[0].allocations:
        if not isinstance(alloc, mybir.MemoryLocationSet):
            continue
        name = alloc.memorylocations[0].name
        if alloc.kind == "ExternalInput":
            if name != partition_name:
                in_names.append(name)
        elif alloc.kind == "ExternalOutput":
            out_names.append(name)
            out_avals.append(
                jax.core.ShapedArray(tuple(alloc.tensor_shape), mybir.dt.np(alloc.dtype))
            )
    n_params = len(in_names)
    n_outs = len(out_avals)
    all_in_names = tuple(in_names) + tuple(out_names)
    if partition_name is not None:
        all_in_names = all_in_names + (partition_name,)
    donate = tuple(range(n_params, n_params + n_outs))

    def _body(*args):
        operands = list(args)
        if partition_name is not None:
            operands.append(bass2jax.partition_id_tensor())
        outs = bass2jax._bass_exec_p.bind(
            *operands,
            out_avals=tuple(out_avals),
            in_names=all_in_names,
            out_names=tuple(out_names),
            lowering_input_output_aliases=(),
            sim_require_finite=True,
            sim_require_nnan=True,
            nc=nc,
        )
        return tuple(outs)

    devices = jax.devices()[:N_CORES]
    assert len(devices) == N_CORES
    mesh = Mesh(np.asarray(devices), ("core",))
    spec = PartitionSpec("core")
    sharded = jax.jit(
        shard_map(
            _body,
            mesh=mesh,
            in_specs=(spec,) * (n_params + n_outs),
            out_specs=(spec,) * n_outs,
            check_rep=False,
        ),
        donate_argnums=donate,
        keep_unused=True,
    )
    sharding = NamedSharding(mesh, spec)
    return {
        "sharded": sharded,
        "sharding": sharding,
        "in_names": in_names,
        "out_names": out_names,
        "out_avals": out_avals,
        "n_params": n_params,
        "n_outs": n_outs,
    }


def _get_runner(nc):
    key = id(nc)
    if key not in _RUNNER_CACHE:
        _RUNNER_CACHE[key] = _make_runner(nc)
    return _RUNNER_CACHE[key]


def _run(nc, in_maps):
    """Execute the kernel across 8 cores; returns per-core output dicts."""
    import numpy as np

    r = _get_runner(nc)
    n_cores = len(in_maps)
    concat_in = [
        np.concatenate([np.asarray(m[name]) for m in in_maps], axis=0)
        for name in r["in_names"]
    ]
    concat_zeros = [
        np.zeros((n_cores * a.shape[0], *a.shape[1:]), a.dtype) for a in r["out_avals"]
    ]
    out_arrs = r["sharded"](*concat_in, *concat_zeros)
    return [
        {
            name: np.asarray(out_arrs[i]).reshape(n_cores, *r["out_avals"][i].shape)[c]
            for i, name in enumerate(r["out_names"])
        }
        for c in range(n_cores)
    ]


def _bench(in_maps, k, m_core, n, reps):
    """Measure steady-state per-GEMM time: the kernel repeated `reps` times
    inside one program, minus the reps=1 program, divided by reps-1. Fixed
    dispatch overhead cancels in the difference. Sets LAST_EXEC_NS."""
    global LAST_EXEC_NS
    import time

    import jax
    import jax.numpy as jnp
    import numpy as np

    runners = {}
    dev_in = None
    for r_reps in (1, reps):
        nc = _get_nc(k, m_core, n, reps=r_reps)
        r = _get_runner(nc)
        runners[r_reps] = r
        if dev_in is None:
            concat_in = [
                np.concatenate([np.asarray(m[name]) for m in in_maps], axis=0)
                for name in r["in_names"]
            ]
            dev_in = [jax.device_put(a, r["sharding"]) for a in concat_in]
            jax.block_until_ready(dev_in)

    def _zeros(r):
        zs = [
            jax.jit(lambda a=a: jnp.zeros(a.shape, a.dtype),
                    out_shardings=r["sharding"])()
            for a in r["out_avals"]
        ]
        jax.block_until_ready(zs)
        return zs

    n_calls = int(os.environ.get("KERNEL_BENCH_CALLS", "3"))

    def _attempt(r_reps):
        # Time n_calls back-to-back dispatches with a single final sync:
        # per-call host/tunnel jitter amortizes across the batch.
        r = runners[r_reps]
        zsets = [_zeros(r) for _ in range(n_calls)]
        t0 = time.perf_counter()
        outs = [r["sharded"](*dev_in, *zs) for zs in zsets]
        jax.block_until_ready(outs)
        return (time.perf_counter() - t0) / n_calls

    for r_reps in (1, reps):  # compile + warmup both programs first
        _attempt(r_reps)

    # Interleaved attempt pairs: slow drift in fixed overhead is common-mode
    # within a pair, so per-pair deltas are far more stable than min-of-each.
    deltas, t1s, tns = [], [], []
    for _ in range(int(os.environ.get("KERNEL_BENCH_TRIES", "6"))):
        t1 = _attempt(1)
        tn = _attempt(reps)
        t1s.append(t1)
        tns.append(tn)
        deltas.append((tn - t1) / (reps - 1))
    per_iter = min(deltas)
    LAST_EXEC_NS = int(per_iter * 1e9)
    print(f"[bench] reps=1: {[f'{a * 1e3:.2f}' for a in t1s]}")
    print(f"[bench] reps={reps}: {[f'{a * 1e3:.2f}' for a in tns]}")
    print(f"[bench] per-GEMM deltas (ms): {[f'{d * 1e3:.3f}' for d in deltas]}")
    print(f"[bench] per-GEMM: {per_iter * 1e3:.3f} ms "
          f"(fixed+1iter: {min(t1s) * 1e3:.3f} ms)")


def kernel(input_, weight, bias):
    global LAST_RESULTS

    input_ = np.asarray(input_, dtype=np.float32)
    weight = np.asarray(weight, dtype=np.float32)
    bias = np.asarray(bias, dtype=np.float32)

    seq, batch, k = input_.shape
    n = weight.shape[0]
    m_full = seq * batch
    m_core = m_full // N_CORES

    variant = _variant()
    nc = _get_nc(k, m_core, n)

    x2 = input_.reshape(m_full, k)
    m_block, dtype, w_scale, out_pm = VARIANT_CFG[variant]
    wp = _w_to_pkn(weight, dtype, scale=w_scale)

    in_maps = []
    for c in range(N_CORES):
        xp = _to_pkm_blocks(x2[c * m_core:(c + 1) * m_core], m_block, dtype)
        in_maps.append({"xb": xp, "wb": wp})

    results = _run(nc, in_maps)
    LAST_RESULTS = results

    bench_reps = int(os.environ.get("KERNEL_BENCH", "0"))
    if bench_reps > 1:
        _bench(in_maps, k, m_core, n, bench_reps)

    per_core = [results[c]["out"] for c in range(N_CORES)]
    if out_pm:
        # device layout [P, m_core//P, n]: row mg*P + p lives at [p, mg, :]
        per_core = [o.transpose(1, 0, 2).reshape(m_core, n) for o in per_core]
    out = np.concatenate(per_core, axis=0)
    if w_scale != 1.0:
        out = out * np.float32(1.0 / w_scale)
    out = out.reshape(seq, batch, n)
    if bias.any():
        out = out + bias
    return out


# revision 33
# speedup vs baseline: 1.2433x; 1.1518x over previous
"""Trainium2 Bass kernel: row-parallel linear  y = einsum('sbk,nk->sbn', x, W) + bias.

Strategy
--------
Full inputs arrive on the host. We flatten (seq, batch) -> M = 8192 rows and
shard M across the 8 NeuronCores (1024 rows each); every core streams the full
weight and computes its [1024, 4096] slice of the output.

The correctness gate is rel_err < 2e-2 (max-abs over max-abs), which a single
reduced-precision GEMM pass meets comfortably:
  - "fp8*" (default fp8e): one e4m3 DoubleRow pass, measured 1.361e-2 rel err,
    ~0.97 ms/GEMM — the PE issue-rate ceiling for DoubleRow (256-deep
    contraction per matmul at ~241 ns for a [256]x[128]x[512] instruction).
  - "bf1": one bf16 pass, measured 8.4e-4 rel err, ~1.61 ms/GEMM (safe
    fallback, PE-bound at ~197 ns per [128]x[128]x[512] matmul).
Host-side quantization makes the device error deterministic: products are
exact in fp8/bf16 and accumulate in fp32 PSUM, so the harness re-measures
the same 1.361e-2 bit-for-bit.

Perf notes (measured on these cores):
  - per-dma_start fixed cost is ~1.5-2 us and DMA count, not bytes, dominated
    the old 3-pass baseline (~2100 DMAs -> 4.8-6.7 ms). Batching W into
    [P, 16, n_chunk] tiles (112 DMAs total) keeps the stream fully hidden.
  - W-pool depth matters: 6 tiles of prefetch absorb tunnel/HBM jitter.
  - All-core wall-clock through PJRT has +-1.5 ms per-call jitter; _bench
    times 3 back-to-back calls per attempt with a single sync and reports
    the min over interleaved (reps=1, reps=N) pair deltas.

Device layout: operands are staged in DRAM with the contraction dim on the
partition axis: x as [n_mb, P, ko, m_block] and W as [P, ko, n] with
k = ko*128 + p, so every SBUF tile load is contiguous-per-partition.

Per core: loop over m-blocks; per m-block the full-K x strip stays resident
in SBUF (loaded as ko-chunked tiles so matmuls start as soon as their chunk
lands and the next block prefetches into spare pool slots); W streams through
once per m-block; 8 PSUM banks hold the (m-strip x n-tile) accumulators
across the whole K loop, evicted once per n-chunk via VectorE.
"""

import os

import numpy as np
import ml_dtypes

BF16 = ml_dtypes.bfloat16
E4M3 = ml_dtypes.float8_e4m3  # TRN semantics: max normal +-240

# Problem shapes (hardcoded per contest contract).
SEQ, BATCH, D_FF, D_MODEL = 2048, 4, 16384, 4096
N_CORES = 8
P = 128

M_FULL = SEQ * BATCH            # 8192
M_CORE = M_FULL // N_CORES      # 1024

MM_N = 512                      # matmul free dim (one fp32 PSUM bank)
KO_LD = 16                      # ko chunks per x load tile

W_SCALE = 128.0                 # fp8: weight pre-scale (power of two, exact)

# Exec-time of the last hardware benchmark (ns), populated when KERNEL_BENCH>0.
LAST_EXEC_NS = None
LAST_RESULTS = None

_BUILD_CACHE = {}
_RUNNER_CACHE = {}


def _build_bf1(k, m_core, n, m_block=512, n_chunk=1024, w_ld=None, reps=1):
    """Single-pass bf16 GEMM: out[m_core, n] = x[m_core, k] @ w[n, k]^T.

    PSUM holds (m_block/128) x (n_chunk/512) fp32 accumulators across the
    full K loop; consecutive matmuls rotate banks. W is streamed once per
    m-block in [P, w_ld, n_chunk] tiles (per-dma_start fixed cost ~1.5us
    dominates below ~1MB transfers, so batch ko planes per DMA); x tiles
    are ko-chunked for fine-grained deps + prefetch; evictions are paired
    into one 512KB output DMA per psum pair."""
    import concourse.mybir as mybir
    import concourse.tile as tile
    from concourse import bacc

    if w_ld is None:
        w_ld = int(os.environ.get("KERNEL_WLD", "4"))
    ko_n = k // P
    n_mb = m_core // m_block
    n_nc = n // n_chunk
    ms_n = m_block // P
    nt_n = n_chunk // MM_N
    n_ld = ko_n // KO_LD
    assert ms_n * nt_n <= 8, "PSUM banks exceeded"
    assert KO_LD % w_ld == 0

    nc = bacc.Bacc(None, target_bir_lowering=False, debug=False)
    xb = nc.declare_dram_parameter("xb", [n_mb, P, ko_n, m_block],
                                   mybir.dt.bfloat16, isOutput=False)
    wb = nc.declare_dram_parameter("wb", [P, ko_n, n],
                                   mybir.dt.bfloat16, isOutput=False)
    out = nc.declare_dram_parameter("out", [m_core, n], mybir.dt.float32,
                                    isOutput=True)

    f32 = mybir.dt.float32
    bf16 = mybir.dt.bfloat16

    with tile.TileContext(nc) as tc:
        with (
            tc.tile_pool(name="xpool", bufs=n_ld + 1) as xpool,
            tc.tile_pool(name="wpool", bufs=4) as wpool,
            tc.tile_pool(name="opool", bufs=2) as opool,
            tc.tile_pool(name="pspool", bufs=8, space="PSUM") as pspool,
        ):
            for rep, mb in ((r_, m_) for r_ in range(reps) for m_ in range(n_mb)):
                xts = []
                for i in range(n_ld):
                    xt = xpool.tile([P, KO_LD, m_block], bf16, tag="xt",
                                    name=f"x_{rep}_{mb}_{i}")
                    nc.sync.dma_start(xt, xb[mb, :, i * KO_LD:(i + 1) * KO_LD, :])
                    xts.append(xt)
                m0 = mb * m_block
                for nc0 in range(n_nc):
                    c0 = nc0 * n_chunk
                    psums = [
                        pspool.tile([P, MM_N], f32, tag="ps",
                                    name=f"ps_{rep}_{mb}_{nc0}_{i}")
                        for i in range(ms_n * nt_n)
                    ]
                    for kw in range(ko_n // w_ld):
                        wt = wpool.tile([P, w_ld, n_chunk], bf16, tag="wt")
                        nc.sync.dma_start(
                            wt, wb[:, kw * w_ld:(kw + 1) * w_ld, c0:c0 + n_chunk])
                        for kj in range(w_ld):
                            ko = kw * w_ld + kj
                            first = ko == 0
                            last = ko == ko_n - 1
                            xt = xts[ko // KO_LD]
                            for ms in range(ms_n):
                                lhs = xt[:, ko % KO_LD, ms * P:(ms + 1) * P]
                                for nt in range(nt_n):
                                    nc.tensor.matmul(
                                        psums[ms * nt_n + nt],
                                        lhs,
                                        wt[:, kj, nt * MM_N:(nt + 1) * MM_N],
                                        start=first,
                                        stop=last,
                                    )
                    for ms in range(ms_n):
                        st = opool.tile([P, nt_n * MM_N], f32, tag="st")
                        for nt in range(nt_n):
                            nc.vector.tensor_copy(
                                out=st[:, nt * MM_N:(nt + 1) * MM_N],
                                in_=psums[ms * nt_n + nt])
                        nc.sync.dma_start(
                            out[m0 + ms * P:m0 + (ms + 1) * P,
                                c0:c0 + nt_n * MM_N],
                            st,
                        )
    nc.compile()
    return nc


def _build_fp8(k, m_core, n, m_block=256, n_chunk=2048, w_ld=None, reps=1,
               merge_out=False, w_bufs=4, split_rings=False, pe_only=False):
    """Single-pass e4m3 GEMM with DoubleRow: each matmul contracts 256 rows
    (2 ko chunks packed per PE cell). Both operands carry a [P, 2, free] AP.
    W arrives pre-scaled by W_SCALE; the host descales the output. W is
    streamed in [P, w_ld, n_chunk] tiles to amortize per-DMA fixed cost."""
    import concourse.mybir as mybir
    import concourse.tile as tile
    from concourse import bacc

    if w_ld is None:
        w_ld = int(os.environ.get("KERNEL_WLD", "4"))
    ko_n = k // P
    n_mb = m_core // m_block
    n_nc = n // n_chunk
    ms_n = m_block // P
    nt_n = n_chunk // MM_N
    n_ld = ko_n // KO_LD
    assert ms_n * nt_n <= 8, "PSUM banks exceeded"
    assert w_ld % 2 == 0 and (KO_LD % w_ld == 0 or w_ld % KO_LD == 0)

    nc = bacc.Bacc(None, target_bir_lowering=False, debug=False)
    xb = nc.declare_dram_parameter("xb", [n_mb, P, ko_n, m_block],
                                   mybir.dt.float8e4, isOutput=False)
    wb = nc.declare_dram_parameter("wb", [P, ko_n, n],
                                   mybir.dt.float8e4, isOutput=False)
    if merge_out:
        # partition-major: out[p, mg, nn] = y[mg*P + p, nn]
        out = nc.declare_dram_parameter("out", [P, m_core // P, n],
                                        mybir.dt.float32, isOutput=True)
    else:
        out = nc.declare_dram_parameter("out", [m_core, n], mybir.dt.float32,
                                        isOutput=True)

    f32 = mybir.dt.float32
    fp8 = mybir.dt.float8e4
    dr = mybir.MatmulPerfMode.DoubleRow

    with tile.TileContext(nc) as tc:
        with (
            tc.tile_pool(name="xpool", bufs=n_ld + 2) as xpool,
            tc.tile_pool(name="wpool", bufs=w_bufs) as wpool,
            tc.tile_pool(name="opool", bufs=2) as opool,
            tc.tile_pool(name="pspool", bufs=8, space="PSUM") as pspool,
        ):
            # x/out DMAs can ride the scalar engine's HWDGE ring so the W
            # stream owns the sync ring's FIFO end-to-end.
            aux = nc.scalar if split_rings else nc.sync
            if pe_only:  # diagnostic: single W tile reused, no streaming
                wt0 = wpool.tile([P, w_ld, n_chunk], fp8, tag="wt")
                nc.sync.dma_start(wt0, wb[:, 0:w_ld, 0:n_chunk])
            for rep, mb in ((r_, m_) for r_ in range(reps) for m_ in range(n_mb)):
                xts = []
                for i in range(n_ld):
                    xt = xpool.tile([P, KO_LD, m_block], fp8, tag="xt",
                                    name=f"x_{rep}_{mb}_{i}")
                    aux.dma_start(xt, xb[mb, :, i * KO_LD:(i + 1) * KO_LD, :])
                    xts.append(xt)
                m0 = mb * m_block
                for nc0 in range(n_nc):
                    c0 = nc0 * n_chunk
                    psums = [
                        pspool.tile([P, MM_N], f32, tag="ps",
                                    name=f"ps_{rep}_{mb}_{nc0}_{i}")
                        for i in range(ms_n * nt_n)
                    ]
                    for kw in range(ko_n // w_ld):
                        if pe_only:
                            wt = wt0
                        else:
                            wt = wpool.tile([P, w_ld, n_chunk], fp8, tag="wt")
                            nc.sync.dma_start(
                                wt, wb[:, kw * w_ld:(kw + 1) * w_ld, c0:c0 + n_chunk])
                        for kj in range(0, w_ld, 2):
                            ko = kw * w_ld + kj
                            first = ko == 0
                            last = ko == ko_n - 2
                            xt = xts[ko // KO_LD]
                            kx = ko % KO_LD
                            for ms in range(ms_n):
                                lhs = xt[:, kx:kx + 2, ms * P:(ms + 1) * P]
                                for nt in range(nt_n):
                                    nc.tensor.matmul(
                                        psums[ms * nt_n + nt],
                                        lhs,
                                        wt[:, kj:kj + 2,
                                           nt * MM_N:(nt + 1) * MM_N],
                                        start=first,
                                        stop=last,
                                        perf_mode=dr,
                                    )
                    if merge_out:
                        msg0 = m0 // P
                        st = opool.tile([P, ms_n, nt_n * MM_N], f32, tag="st")
                        for ms in range(ms_n):
                            for nt in range(nt_n):
                                nc.vector.tensor_copy(
                                    out=st[:, ms, nt * MM_N:(nt + 1) * MM_N],
                                    in_=psums[ms * nt_n + nt])
                        nc.sync.dma_start(
                            out[:, msg0:msg0 + ms_n, c0:c0 + nt_n * MM_N],
                            st,
                        )
                    else:
                        for ms in range(ms_n):
                            st = opool.tile([P, nt_n * MM_N], f32, tag="st")
                            for nt in range(nt_n):
                                nc.vector.tensor_copy(
                                    out=st[:, nt * MM_N:(nt + 1) * MM_N],
                                    in_=psums[ms * nt_n + nt])
                            aux.dma_start(
                                out[m0 + ms * P:m0 + (ms + 1) * P,
                                    c0:c0 + nt_n * MM_N],
                                st,
                            )
    nc.compile()
    return nc


def _build_bf1_nomm(k, m_core, n, m_block=512, n_chunk=1024, reps=1):
    """Diagnostic: bf1's exact DMA stream with no matmuls (times pure DMA)."""
    import concourse.mybir as mybir
    import concourse.tile as tile
    from concourse import bacc

    ko_n = k // P
    n_mb = m_core // m_block
    n_nc = n // n_chunk
    n_ld = ko_n // KO_LD

    nc = bacc.Bacc(None, target_bir_lowering=False, debug=False)
    xb = nc.declare_dram_parameter("xb", [n_mb, P, ko_n, m_block],
                                   mybir.dt.bfloat16, isOutput=False)
    wb = nc.declare_dram_parameter("wb", [P, ko_n, n],
                                   mybir.dt.bfloat16, isOutput=False)
    out = nc.declare_dram_parameter("out", [m_core, n], mybir.dt.float32,
                                    isOutput=True)
    bf16 = mybir.dt.bfloat16
    with tile.TileContext(nc) as tc:
        with (
            tc.tile_pool(name="xpool", bufs=n_ld + 2) as xpool,
            tc.tile_pool(name="wpool", bufs=6) as wpool,
        ):
            for rep, mb in ((r_, m_) for r_ in range(reps) for m_ in range(n_mb)):
                for i in range(n_ld):
                    xt = xpool.tile([P, KO_LD, m_block], bf16, tag="xt",
                                    name=f"x_{rep}_{mb}_{i}")
                    nc.sync.dma_start(xt, xb[mb, :, i * KO_LD:(i + 1) * KO_LD, :])
                for nc0 in range(n_nc):
                    c0 = nc0 * n_chunk
                    for ko in range(ko_n):
                        wt = wpool.tile([P, n_chunk], bf16, tag="wt")
                        nc.sync.dma_start(wt, wb[:, ko, c0:c0 + n_chunk])
    nc.compile()
    return nc


def _build_bf1_nodma(k, m_core, n, m_block=512, n_chunk=1024, reps=1):
    """Diagnostic: bf1's exact matmul stream with W loaded once (times pure PE)."""
    import concourse.mybir as mybir
    import concourse.tile as tile
    from concourse import bacc

    ko_n = k // P
    n_mb = m_core // m_block
    n_nc = n // n_chunk
    ms_n = m_block // P
    nt_n = n_chunk // MM_N
    n_ld = ko_n // KO_LD

    nc = bacc.Bacc(None, target_bir_lowering=False, debug=False)
    xb = nc.declare_dram_parameter("xb", [n_mb, P, ko_n, m_block],
                                   mybir.dt.bfloat16, isOutput=False)
    wb = nc.declare_dram_parameter("wb", [P, ko_n, n],
                                   mybir.dt.bfloat16, isOutput=False)
    out = nc.declare_dram_parameter("out", [m_core, n], mybir.dt.float32,
                                    isOutput=True)
    f32 = mybir.dt.float32
    bf16 = mybir.dt.bfloat16
    with tile.TileContext(nc) as tc:
        with (
            tc.tile_pool(name="xpool", bufs=2) as xpool,
            tc.tile_pool(name="wpool", bufs=1) as wpool,
            tc.tile_pool(name="opool", bufs=4) as opool,
            tc.tile_pool(name="pspool", bufs=8, space="PSUM") as pspool,
        ):
            xt = xpool.tile([P, KO_LD, m_block], bf16, tag="xt")
            nc.sync.dma_start(xt, xb[0, :, 0:KO_LD, :])
            wt = wpool.tile([P, n_chunk], bf16, tag="wt")
            nc.sync.dma_start(wt, wb[:, 0, 0:n_chunk])
            for rep, mb in ((r_, m_) for r_ in range(reps) for m_ in range(n_mb)):
                m0 = mb * m_block
                for nc0 in range(n_nc):
                    c0 = nc0 * n_chunk
                    psums = [
                        pspool.tile([P, MM_N], f32, tag="ps",
                                    name=f"ps_{rep}_{mb}_{nc0}_{i}")
                        for i in range(ms_n * nt_n)
                    ]
                    for ko in range(ko_n):
                        first = ko == 0
                        last = ko == ko_n - 1
                        kj = ko % KO_LD
                        for ms in range(ms_n):
                            lhs = xt[:, kj, ms * P:(ms + 1) * P]
                            for nt in range(nt_n):
                                nc.tensor.matmul(
                                    psums[ms * nt_n + nt],
                                    lhs,
                                    wt[:, nt * MM_N:(nt + 1) * MM_N],
                                    start=first,
                                    stop=last,
                                )
                    for ms in range(ms_n):
                        for nt in range(nt_n):
                            st = opool.tile([P, MM_N], f32, tag="st")
                            nc.vector.tensor_copy(out=st, in_=psums[ms * nt_n + nt])
                            nc.sync.dma_start(
                                out[m0 + ms * P:m0 + (ms + 1) * P,
                                    c0 + nt * MM_N:c0 + (nt + 1) * MM_N],
                                st,
                            )
    nc.compile()
    return nc


_BUILDERS = {
    "bf1": _build_bf1,
    "fp8": _build_fp8,
    "fp8b": lambda k, m, n, **kw: _build_fp8(k, m, n, m_block=512, n_chunk=1024,
                                             w_ld=8, **kw),
    "fp8c": lambda k, m, n, **kw: _build_fp8(k, m, n, m_block=512, n_chunk=1024,
                                             w_ld=16, **kw),
    "fp8d": lambda k, m, n, **kw: _build_fp8(k, m, n, m_block=512, n_chunk=1024,
                                             w_ld=32, w_bufs=2, merge_out=True,
                                             **kw),
    "fp8e": lambda k, m, n, **kw: _build_fp8(k, m, n, m_block=512, n_chunk=1024,
                                             w_ld=16, w_bufs=6, **kw),
    "fp8f": lambda k, m, n, **kw: _build_fp8(k, m, n, m_block=512, n_chunk=1024,
                                             w_ld=16, w_bufs=7, **kw),
    "fp8g": lambda k, m, n, **kw: _build_fp8(k, m, n, m_block=256, n_chunk=2048,
                                             w_ld=16, w_bufs=4, **kw),
    "fp8h": lambda k, m, n, **kw: _build_fp8(k, m, n, m_block=512, n_chunk=1024,
                                             w_ld=16, w_bufs=6, split_rings=True,
                                             **kw),
    "fp8_pe": lambda k, m, n, **kw: _build_fp8(k, m, n, m_block=512,
                                               n_chunk=1024, w_ld=16, w_bufs=6,
                                               pe_only=True, **kw),
    "fp8i": lambda k, m, n, **kw: _build_fp8(k, m, n, m_block=512, n_chunk=1024,
                                             w_ld=8, w_bufs=12, split_rings=True,
                                             **kw),
    "bf1_nomm": _build_bf1_nomm,
    "bf1_nodma": _build_bf1_nodma,
}

# variant -> (m_block for host x layout, operand dtype, W pre-scale,
#             out is partition-major [P, m_core//P, n])
VARIANT_CFG = {
    "bf1": (512, BF16, 1.0, False),
    "fp8": (256, E4M3, W_SCALE, False),
    "fp8b": (512, E4M3, W_SCALE, False),
    "fp8c": (512, E4M3, W_SCALE, False),
    "fp8d": (512, E4M3, W_SCALE, True),
    "fp8e": (512, E4M3, W_SCALE, False),
    "fp8f": (512, E4M3, W_SCALE, False),
    "fp8g": (256, E4M3, W_SCALE, False),
    "fp8h": (512, E4M3, W_SCALE, False),
    "fp8i": (512, E4M3, W_SCALE, False),
    "fp8_pe": (512, E4M3, W_SCALE, False),
    "bf1_nomm": (512, BF16, 1.0, False),
    "bf1_nodma": (512, BF16, 1.0, False),
}


def _variant():
    return os.environ.get("KERNEL_VARIANT", "fp8e")


def _get_nc(k, m_core, n, **kw):
    variant = _variant()
    key = (variant, k, m_core, n, tuple(sorted(kw.items())))
    if key not in _BUILD_CACHE:
        _BUILD_CACHE[key] = _BUILDERS[variant](k, m_core, n, **kw)
    return _BUILD_CACHE[key]


def _to_pkm_blocks(a, m_block, dtype):
    """[rows, k] fp32 -> contiguous [n_mb, P, ko_n, m_block] in `dtype`
    (k = ko*128 + p)."""
    rows, k = a.shape
    n_mb = rows // m_block
    ko_n = k // P
    a = a.astype(dtype)
    a = a.reshape(n_mb, m_block, ko_n, P).transpose(0, 3, 2, 1)
    return np.ascontiguousarray(a)


def _w_to_pkn(w, dtype, scale=1.0):
    """[n, k] fp32 -> contiguous [P, ko_n, n] in `dtype`."""
    n, k = w.shape
    ko_n = k // P
    if scale != 1.0:
        w = w * np.float32(scale)
    w = w.astype(dtype)
    w = w.reshape(n, ko_n, P).transpose(2, 1, 0)
    return np.ascontiguousarray(w)


def _make_runner(nc):
    """Build the sharded PJRT executor for `nc` across the 8 cores.

    Mirrors concourse.bass2jax.run_bass_via_pjrt, but returns a reusable
    closure so repeated calls share one jit cache and inputs can stay
    device-resident for benchmarking.
    """
    import jax
    import concourse.mybir as mybir
    from concourse import bass2jax
    from jax.experimental.shard_map import shard_map
    from jax.sharding import Mesh, NamedSharding, PartitionSpec

    bass2jax.install_neuronx_cc_hook()

    partition_name = nc.partition_id_tensor.name if nc.partition_id_tensor else None
    assert nc.dbg_addr is None

    in_names, out_names, out_avals = [], [], []
    for alloc in nc.m.functions[0].allocations:
        if not isinstance(alloc, mybir.MemoryLocationSet):
            continue
        name = alloc.memorylocations[0].name
        if alloc.kind == "ExternalInput":
            if name != partition_name:
                in_names.append(name)
        elif alloc.kind == "ExternalOutput":
            out_names.append(name)
            out_avals.append(
                jax.core.ShapedArray(tuple(alloc.tensor_shape), mybir.dt.np(alloc.dtype))
            )
    n_params = len(in_names)
    n_outs = len(out_avals)
    all_in_names = tuple(in_names) + tuple(out_names)
    if partition_name is not None:
        all_in_names = all_in_names + (partition_name,)
    donate = tuple(range(n_params, n_params + n_outs))

    def _body(*args):
        operands = list(args)
        if partition_name is not None:
            operands.append(bass2jax.partition_id_tensor())
        outs = bass2jax._bass_exec_p.bind(
            *operands,
            out_avals=tuple(out_avals),
            in_names=all_in_names,
            out_names=tuple(out_names),
            lowering_input_output_aliases=(),
            sim_require_finite=True,
            sim_require_nnan=True,
            nc=nc,
        )
        return tuple(outs)

    devices = jax.devices()[:N_CORES]
    assert len(devices) == N_CORES
    mesh = Mesh(np.asarray(devices), ("core",))
    spec = PartitionSpec("core")
    sharded = jax.jit(
        shard_map(
            _body,
            mesh=mesh,
            in_specs=(spec,) * (n_params + n_outs),
            out_specs=(spec,) * n_outs,
            check_rep=False,
        ),
        donate_argnums=donate,
        keep_unused=True,
    )
    sharding = NamedSharding(mesh, spec)
    return {
        "sharded": sharded,
        "sharding": sharding,
        "in_names": in_names,
        "out_names": out_names,
        "out_avals": out_avals,
        "n_params": n_params,
        "n_outs": n_outs,
    }


def _get_runner(nc):
    key = id(nc)
    if key not in _RUNNER_CACHE:
        _RUNNER_CACHE[key] = _make_runner(nc)
    return _RUNNER_CACHE[key]


def _run(nc, in_maps):
    """Execute the kernel across 8 cores; returns per-core output dicts."""
    import numpy as np

    r = _get_runner(nc)
    n_cores = len(in_maps)
    concat_in = [
        np.concatenate([np.asarray(m[name]) for m in in_maps], axis=0)
        for name in r["in_names"]
    ]
    concat_zeros = [
        np.zeros((n_cores * a.shape[0], *a.shape[1:]), a.dtype) for a in r["out_avals"]
    ]
    out_arrs = r["sharded"](*concat_in, *concat_zeros)
    return [
        {
            name: np.asarray(out_arrs[i]).reshape(n_cores, *r["out_avals"][i].shape)[c]
            for i, name in enumerate(r["out_names"])
        }
        for c in range(n_cores)
    ]


def _bench(in_maps, k, m_core, n, reps):
    """Measure steady-state per-GEMM time: the kernel repeated `reps` times
    inside one program, minus the reps=1 program, divided by reps-1. Fixed
    dispatch overhead cancels in the difference. Sets LAST_EXEC_NS."""
    global LAST_EXEC_NS
    import time

    import jax
    import jax.numpy as jnp
    import numpy as np

    runners = {}
    dev_in = None
    for r_reps in (1, reps):
        nc = _get_nc(k, m_core, n, reps=r_reps)
        r = _get_runner(nc)
        runners[r_reps] = r
        if dev_in is None:
            concat_in = [
                np.concatenate([np.asarray(m[name]) for m in in_maps], axis=0)
                for name in r["in_names"]
            ]
            dev_in = [jax.device_put(a, r["sharding"]) for a in concat_in]
            jax.block_until_ready(dev_in)

    def _zeros(r):
        zs = [
            jax.jit(lambda a=a: jnp.zeros(a.shape, a.dtype),
                    out_shardings=r["sharding"])()
            for a in r["out_avals"]
        ]
        jax.block_until_ready(zs)
        return zs

    n_calls = int(os.environ.get("KERNEL_BENCH_CALLS", "3"))

    def _attempt(r_reps):
        # Time n_calls back-to-back dispatches with a single final sync:
        # per-call host/tunnel jitter amortizes across the batch.
        r = runners[r_reps]
        zsets = [_zeros(r) for _ in range(n_calls)]
        t0 = time.perf_counter()
        outs = [r["sharded"](*dev_in, *zs) for zs in zsets]
        jax.block_until_ready(outs)
        return (time.perf_counter() - t0) / n_calls

    for r_reps in (1, reps):  # compile + warmup both programs first
        _attempt(r_reps)

    # Interleaved attempt pairs: slow drift in fixed overhead is common-mode
    # within a pair, so per-pair deltas are far more stable than min-of-each.
    deltas, t1s, tns = [], [], []
    for _ in range(int(os.environ.get("KERNEL_BENCH_TRIES", "6"))):
        t1 = _attempt(1)
        tn = _attempt(reps)
        t1s.append(t1)
        tns.append(tn)
        deltas.append((tn - t1) / (reps - 1))
    per_iter = min(deltas)
    LAST_EXEC_NS = int(per_iter * 1e9)
    print(f"[bench] reps=1: {[f'{a * 1e3:.2f}' for a in t1s]}")
    print(f"[bench] reps={reps}: {[f'{a * 1e3:.2f}' for a in tns]}")
    print(f"[bench] per-GEMM deltas (ms): {[f'{d * 1e3:.3f}' for d in deltas]}")
    print(f"[bench] per-GEMM: {per_iter * 1e3:.3f} ms "
          f"(fixed+1iter: {min(t1s) * 1e3:.3f} ms)")


def kernel(input_, weight, bias):
    global LAST_RESULTS

    input_ = np.asarray(input_, dtype=np.float32)
    weight = np.asarray(weight, dtype=np.float32)
    bias = np.asarray(bias, dtype=np.float32)

    seq, batch, k = input_.shape
    n = weight.shape[0]
    m_full = seq * batch
    m_core = m_full // N_CORES

    variant = _variant()
    nc = _get_nc(k, m_core, n)

    x2 = input_.reshape(m_full, k)
    m_block, dtype, w_scale, out_pm = VARIANT_CFG[variant]
    wp = _w_to_pkn(weight, dtype, scale=w_scale)

    in_maps = []
    for c in range(N_CORES):
        xp = _to_pkm_blocks(x2[c * m_core:(c + 1) * m_core], m_block, dtype)
        in_maps.append({"xb": xp, "wb": wp})

    results = _run(nc, in_maps)
    LAST_RESULTS = results

    bench_reps = int(os.environ.get("KERNEL_BENCH", "0"))
    if bench_reps > 1:
        _bench(in_maps, k, m_core, n, bench_reps)

    per_core = [results[c]["out"] for c in range(N_CORES)]
    if out_pm:
        # device layout [P, m_core//P, n]: row mg*P + p lives at [p, mg, :]
        per_core = [o.transpose(1, 0, 2).reshape(m_core, n) for o in per_core]
    out = np.concatenate(per_core, axis=0)
    if w_scale != 1.0:
        out = out * np.float32(1.0 / w_scale)
    out = out.reshape(seq, batch, n)
    if bias.any():
        out = out + bias
    return out


# revision 43
# speedup vs baseline: 1.9444x; 1.5639x over previous
"""Trainium2 Bass kernel: row-parallel linear  y = einsum('sbk,nk->sbn', x, W) + bias.

Strategy
--------
Full inputs arrive on the host. We flatten (seq, batch) -> M = 8192 rows and
shard M across the 8 NeuronCores (1024 rows each); every core streams the full
weight and computes its [1024, 4096] slice of the output.

The correctness gate is rel_err < 2e-2 (max-abs over max-abs), which a single
reduced-precision GEMM pass meets comfortably:
  - "fp8*" (default fp8e): one e4m3 DoubleRow pass, measured 1.361e-2 rel err,
    ~0.97 ms/GEMM — the PE issue-rate ceiling for DoubleRow (256-deep
    contraction per matmul at ~241 ns for a [256]x[128]x[512] instruction).
  - "bf1": one bf16 pass, measured 8.4e-4 rel err, ~1.61 ms/GEMM (safe
    fallback, PE-bound at ~197 ns per [128]x[128]x[512] matmul).
Host-side quantization makes the device error deterministic: products are
exact in fp8/bf16 and accumulate in fp32 PSUM, so the harness re-measures
the same 1.361e-2 bit-for-bit.

Perf notes (measured on these cores):
  - per-dma_start fixed cost is ~1.5-2 us and DMA count, not bytes, dominated
    the old 3-pass baseline (~2100 DMAs -> 4.8-6.7 ms). Batching W into
    [P, 16, n_chunk] tiles (112 DMAs total) keeps the stream fully hidden.
  - W-pool depth matters: 6 tiles of prefetch absorb tunnel/HBM jitter.
  - All-core wall-clock through PJRT has +-1.5 ms per-call jitter; _bench
    times 3 back-to-back calls per attempt with a single sync and reports
    the min over interleaved (reps=1, reps=N) pair deltas.

Device layout: operands are staged in DRAM with the contraction dim on the
partition axis: x as [n_mb, P, ko, m_block] and W as [P, ko, n] with
k = ko*128 + p, so every SBUF tile load is contiguous-per-partition.

Per core: loop over m-blocks; per m-block the full-K x strip stays resident
in SBUF (loaded as ko-chunked tiles so matmuls start as soon as their chunk
lands and the next block prefetches into spare pool slots); W streams through
once per m-block; 8 PSUM banks hold the (m-strip x n-tile) accumulators
across the whole K loop, evicted once per n-chunk via VectorE.
"""

import os

import numpy as np
import ml_dtypes

BF16 = ml_dtypes.bfloat16
E4M3 = ml_dtypes.float8_e4m3  # TRN semantics: max normal +-240

# Problem shapes (hardcoded per contest contract).
SEQ, BATCH, D_FF, D_MODEL = 2048, 4, 16384, 4096
N_CORES = 8
P = 128

M_FULL = SEQ * BATCH            # 8192
M_CORE = M_FULL // N_CORES      # 1024

MM_N = 512                      # matmul free dim (one fp32 PSUM bank)
KO_LD = 16                      # ko chunks per x load tile

W_SCALE = 128.0                 # fp8: weight pre-scale (power of two, exact)

# Exec-time of the last hardware benchmark (ns), populated when KERNEL_BENCH>0.
LAST_EXEC_NS = None
LAST_RESULTS = None

_BUILD_CACHE = {}
_RUNNER_CACHE = {}


def _build_bf1(k, m_core, n, m_block=512, n_chunk=1024, w_ld=None, reps=1):
    """Single-pass bf16 GEMM: out[m_core, n] = x[m_core, k] @ w[n, k]^T.

    PSUM holds (m_block/128) x (n_chunk/512) fp32 accumulators across the
    full K loop; consecutive matmuls rotate banks. W is streamed once per
    m-block in [P, w_ld, n_chunk] tiles (per-dma_start fixed cost ~1.5us
    dominates below ~1MB transfers, so batch ko planes per DMA); x tiles
    are ko-chunked for fine-grained deps + prefetch; evictions are paired
    into one 512KB output DMA per psum pair."""
    import concourse.mybir as mybir
    import concourse.tile as tile
    from concourse import bacc

    if w_ld is None:
        w_ld = int(os.environ.get("KERNEL_WLD", "4"))
    ko_n = k // P
    n_mb = m_core // m_block
    n_nc = n // n_chunk
    ms_n = m_block // P
    nt_n = n_chunk // MM_N
    n_ld = ko_n // KO_LD
    assert ms_n * nt_n <= 8, "PSUM banks exceeded"
    assert KO_LD % w_ld == 0

    nc = bacc.Bacc(None, target_bir_lowering=False, debug=False)
    xb = nc.declare_dram_parameter("xb", [n_mb, P, ko_n, m_block],
                                   mybir.dt.bfloat16, isOutput=False)
    wb = nc.declare_dram_parameter("wb", [P, ko_n, n],
                                   mybir.dt.bfloat16, isOutput=False)
    out = nc.declare_dram_parameter("out", [m_core, n], mybir.dt.float32,
                                    isOutput=True)

    f32 = mybir.dt.float32
    bf16 = mybir.dt.bfloat16

    with tile.TileContext(nc) as tc:
        with (
            tc.tile_pool(name="xpool", bufs=n_ld + 1) as xpool,
            tc.tile_pool(name="wpool", bufs=4) as wpool,
            tc.tile_pool(name="opool", bufs=2) as opool,
            tc.tile_pool(name="pspool", bufs=8, space="PSUM") as pspool,
        ):
            for rep, mb in ((r_, m_) for r_ in range(reps) for m_ in range(n_mb)):
                xts = []
                for i in range(n_ld):
                    xt = xpool.tile([P, KO_LD, m_block], bf16, tag="xt",
                                    name=f"x_{rep}_{mb}_{i}")
                    nc.sync.dma_start(xt, xb[mb, :, i * KO_LD:(i + 1) * KO_LD, :])
                    xts.append(xt)
                m0 = mb * m_block
                for nc0 in range(n_nc):
                    c0 = nc0 * n_chunk
                    psums = [
                        pspool.tile([P, MM_N], f32, tag="ps",
                                    name=f"ps_{rep}_{mb}_{nc0}_{i}")
                        for i in range(ms_n * nt_n)
                    ]
                    for kw in range(ko_n // w_ld):
                        wt = wpool.tile([P, w_ld, n_chunk], bf16, tag="wt")
                        nc.sync.dma_start(
                            wt, wb[:, kw * w_ld:(kw + 1) * w_ld, c0:c0 + n_chunk])
                        for kj in range(w_ld):
                            ko = kw * w_ld + kj
                            first = ko == 0
                            last = ko == ko_n - 1
                            xt = xts[ko // KO_LD]
                            for ms in range(ms_n):
                                lhs = xt[:, ko % KO_LD, ms * P:(ms + 1) * P]
                                for nt in range(nt_n):
                                    nc.tensor.matmul(
                                        psums[ms * nt_n + nt],
                                        lhs,
                                        wt[:, kj, nt * MM_N:(nt + 1) * MM_N],
                                        start=first,
                                        stop=last,
                                    )
                    for ms in range(ms_n):
                        st = opool.tile([P, nt_n * MM_N], f32, tag="st")
                        for nt in range(nt_n):
                            nc.vector.tensor_copy(
                                out=st[:, nt * MM_N:(nt + 1) * MM_N],
                                in_=psums[ms * nt_n + nt])
                        nc.sync.dma_start(
                            out[m0 + ms * P:m0 + (ms + 1) * P,
                                c0:c0 + nt_n * MM_N],
                            st,
                        )
    nc.compile()
    return nc


def _build_fp8(k, m_core, n, m_block=256, n_chunk=2048, w_ld=None, reps=1,
               merge_out=False, w_bufs=4, split_rings=False, pe_only=False,
               w_tiled=False):
    """Single-pass e4m3 GEMM with DoubleRow: each matmul contracts 256 rows
    (2 ko chunks packed per PE cell). Both operands carry a [P, 2, free] AP.
    W arrives pre-scaled by W_SCALE; the host descales the output. W is
    streamed in [P, w_ld, n_chunk] tiles to amortize per-DMA fixed cost."""
    import concourse.mybir as mybir
    import concourse.tile as tile
    from concourse import bacc

    if w_ld is None:
        w_ld = int(os.environ.get("KERNEL_WLD", "4"))
    ko_n = k // P
    n_mb = m_core // m_block
    n_nc = n // n_chunk
    ms_n = m_block // P
    nt_n = n_chunk // MM_N
    n_ld = ko_n // KO_LD
    assert ms_n * nt_n <= 8, "PSUM banks exceeded"
    assert w_ld % 2 == 0 and (KO_LD % w_ld == 0 or w_ld % KO_LD == 0)

    nc = bacc.Bacc(None, target_bir_lowering=False, debug=False)
    xb = nc.declare_dram_parameter("xb", [n_mb, P, ko_n, m_block],
                                   mybir.dt.float8e4, isOutput=False)
    if w_tiled:
        # tile-contiguous: one 8KB descriptor per partition per W DMA,
        # instead of w_ld scattered 1KB chunks (descriptor-rate bound);
        # flat leading tile index (nc0 * kw_n + kw) — single-int indexing
        # matches the proven xb[mb] DMA pattern
        wb = nc.declare_dram_parameter(
            "wb", [n_nc * (ko_n // w_ld), P, w_ld, n_chunk],
            mybir.dt.float8e4, isOutput=False)
    else:
        wb = nc.declare_dram_parameter("wb", [P, ko_n, n],
                                       mybir.dt.float8e4, isOutput=False)
    if merge_out:
        # partition-major: out[p, mg, nn] = y[mg*P + p, nn]
        out = nc.declare_dram_parameter("out", [P, m_core // P, n],
                                        mybir.dt.float32, isOutput=True)
    else:
        out = nc.declare_dram_parameter("out", [m_core, n], mybir.dt.float32,
                                        isOutput=True)

    f32 = mybir.dt.float32
    fp8 = mybir.dt.float8e4
    dr = mybir.MatmulPerfMode.DoubleRow

    with tile.TileContext(nc) as tc:
        with (
            tc.tile_pool(name="xpool", bufs=n_ld + 2) as xpool,
            tc.tile_pool(name="wpool", bufs=w_bufs) as wpool,
            tc.tile_pool(name="opool", bufs=2) as opool,
            tc.tile_pool(name="pspool", bufs=8, space="PSUM") as pspool,
        ):
            # x/out DMAs can ride the scalar engine's HWDGE ring so the W
            # stream owns the sync ring's FIFO end-to-end.
            aux = nc.scalar if split_rings else nc.sync
            if pe_only:  # diagnostic: single W tile reused, no streaming
                wt0 = wpool.tile([P, w_ld, n_chunk], fp8, tag="wt")
                nc.sync.dma_start(wt0, wb[:, 0:w_ld, 0:n_chunk])
            for rep, mb in ((r_, m_) for r_ in range(reps) for m_ in range(n_mb)):
                xts = []
                for i in range(n_ld):
                    xt = xpool.tile([P, KO_LD, m_block], fp8, tag="xt",
                                    name=f"x_{rep}_{mb}_{i}")
                    aux.dma_start(xt, xb[mb, :, i * KO_LD:(i + 1) * KO_LD, :])
                    xts.append(xt)
                m0 = mb * m_block
                for nc0 in range(n_nc):
                    c0 = nc0 * n_chunk
                    psums = [
                        pspool.tile([P, MM_N], f32, tag="ps",
                                    name=f"ps_{rep}_{mb}_{nc0}_{i}")
                        for i in range(ms_n * nt_n)
                    ]
                    for kw in range(ko_n // w_ld):
                        if pe_only:
                            wt = wt0
                        else:
                            wt = wpool.tile([P, w_ld, n_chunk], fp8, tag="wt")
                            if w_tiled:
                                nc.sync.dma_start(
                                    wt, wb[nc0 * (ko_n // w_ld) + kw, :, :, :])
                            else:
                                nc.sync.dma_start(
                                    wt, wb[:, kw * w_ld:(kw + 1) * w_ld,
                                           c0:c0 + n_chunk])
                        for kj in range(0, w_ld, 2):
                            ko = kw * w_ld + kj
                            first = ko == 0
                            last = ko == ko_n - 2
                            xt = xts[ko // KO_LD]
                            kx = ko % KO_LD
                            for ms in range(ms_n):
                                lhs = xt[:, kx:kx + 2, ms * P:(ms + 1) * P]
                                for nt in range(nt_n):
                                    nc.tensor.matmul(
                                        psums[ms * nt_n + nt],
                                        lhs,
                                        wt[:, kj:kj + 2,
                                           nt * MM_N:(nt + 1) * MM_N],
                                        start=first,
                                        stop=last,
                                        perf_mode=dr,
                                    )
                    if merge_out:
                        msg0 = m0 // P
                        st = opool.tile([P, ms_n, nt_n * MM_N], f32, tag="st")
                        for ms in range(ms_n):
                            for nt in range(nt_n):
                                nc.vector.tensor_copy(
                                    out=st[:, ms, nt * MM_N:(nt + 1) * MM_N],
                                    in_=psums[ms * nt_n + nt])
                        nc.sync.dma_start(
                            out[:, msg0:msg0 + ms_n, c0:c0 + nt_n * MM_N],
                            st,
                        )
                    else:
                        for ms in range(ms_n):
                            st = opool.tile([P, nt_n * MM_N], f32, tag="st")
                            for nt in range(nt_n):
                                nc.vector.tensor_copy(
                                    out=st[:, nt * MM_N:(nt + 1) * MM_N],
                                    in_=psums[ms * nt_n + nt])
                            aux.dma_start(
                                out[m0 + ms * P:m0 + (ms + 1) * P,
                                    c0:c0 + nt_n * MM_N],
                                st,
                            )
    nc.compile()
    return nc


def _build_bf1_nomm(k, m_core, n, m_block=512, n_chunk=1024, reps=1):
    """Diagnostic: bf1's exact DMA stream with no matmuls (times pure DMA)."""
    import concourse.mybir as mybir
    import concourse.tile as tile
    from concourse import bacc

    ko_n = k // P
    n_mb = m_core // m_block
    n_nc = n // n_chunk
    n_ld = ko_n // KO_LD

    nc = bacc.Bacc(None, target_bir_lowering=False, debug=False)
    xb = nc.declare_dram_parameter("xb", [n_mb, P, ko_n, m_block],
                                   mybir.dt.bfloat16, isOutput=False)
    wb = nc.declare_dram_parameter("wb", [P, ko_n, n],
                                   mybir.dt.bfloat16, isOutput=False)
    out = nc.declare_dram_parameter("out", [m_core, n], mybir.dt.float32,
                                    isOutput=True)
    bf16 = mybir.dt.bfloat16
    with tile.TileContext(nc) as tc:
        with (
            tc.tile_pool(name="xpool", bufs=n_ld + 2) as xpool,
            tc.tile_pool(name="wpool", bufs=6) as wpool,
        ):
            for rep, mb in ((r_, m_) for r_ in range(reps) for m_ in range(n_mb)):
                for i in range(n_ld):
                    xt = xpool.tile([P, KO_LD, m_block], bf16, tag="xt",
                                    name=f"x_{rep}_{mb}_{i}")
                    nc.sync.dma_start(xt, xb[mb, :, i * KO_LD:(i + 1) * KO_LD, :])
                for nc0 in range(n_nc):
                    c0 = nc0 * n_chunk
                    for ko in range(ko_n):
                        wt = wpool.tile([P, n_chunk], bf16, tag="wt")
                        nc.sync.dma_start(wt, wb[:, ko, c0:c0 + n_chunk])
    nc.compile()
    return nc


def _build_bf1_nodma(k, m_core, n, m_block=512, n_chunk=1024, reps=1):
    """Diagnostic: bf1's exact matmul stream with W loaded once (times pure PE)."""
    import concourse.mybir as mybir
    import concourse.tile as tile
    from concourse import bacc

    ko_n = k // P
    n_mb = m_core // m_block
    n_nc = n // n_chunk
    ms_n = m_block // P
    nt_n = n_chunk // MM_N
    n_ld = ko_n // KO_LD

    nc = bacc.Bacc(None, target_bir_lowering=False, debug=False)
    xb = nc.declare_dram_parameter("xb", [n_mb, P, ko_n, m_block],
                                   mybir.dt.bfloat16, isOutput=False)
    wb = nc.declare_dram_parameter("wb", [P, ko_n, n],
                                   mybir.dt.bfloat16, isOutput=False)
    out = nc.declare_dram_parameter("out", [m_core, n], mybir.dt.float32,
                                    isOutput=True)
    f32 = mybir.dt.float32
    bf16 = mybir.dt.bfloat16
    with tile.TileContext(nc) as tc:
        with (
            tc.tile_pool(name="xpool", bufs=2) as xpool,
            tc.tile_pool(name="wpool", bufs=1) as wpool,
            tc.tile_pool(name="opool", bufs=4) as opool,
            tc.tile_pool(name="pspool", bufs=8, space="PSUM") as pspool,
        ):
            xt = xpool.tile([P, KO_LD, m_block], bf16, tag="xt")
            nc.sync.dma_start(xt, xb[0, :, 0:KO_LD, :])
            wt = wpool.tile([P, n_chunk], bf16, tag="wt")
            nc.sync.dma_start(wt, wb[:, 0, 0:n_chunk])
            for rep, mb in ((r_, m_) for r_ in range(reps) for m_ in range(n_mb)):
                m0 = mb * m_block
                for nc0 in range(n_nc):
                    c0 = nc0 * n_chunk
                    psums = [
                        pspool.tile([P, MM_N], f32, tag="ps",
                                    name=f"ps_{rep}_{mb}_{nc0}_{i}")
                        for i in range(ms_n * nt_n)
                    ]
                    for ko in range(ko_n):
                        first = ko == 0
                        last = ko == ko_n - 1
                        kj = ko % KO_LD
                        for ms in range(ms_n):
                            lhs = xt[:, kj, ms * P:(ms + 1) * P]
                            for nt in range(nt_n):
                                nc.tensor.matmul(
                                    psums[ms * nt_n + nt],
                                    lhs,
                                    wt[:, nt * MM_N:(nt + 1) * MM_N],
                                    start=first,
                                    stop=last,
                                )
                    for ms in range(ms_n):
                        for nt in range(nt_n):
                            st = opool.tile([P, MM_N], f32, tag="st")
                            nc.vector.tensor_copy(out=st, in_=psums[ms * nt_n + nt])
                            nc.sync.dma_start(
                                out[m0 + ms * P:m0 + (ms + 1) * P,
                                    c0 + nt * MM_N:c0 + (nt + 1) * MM_N],
                                st,
                            )
    nc.compile()
    return nc


_BUILDERS = {
    "bf1": _build_bf1,
    "fp8": _build_fp8,
    "fp8b": lambda k, m, n, **kw: _build_fp8(k, m, n, m_block=512, n_chunk=1024,
                                             w_ld=8, **kw),
    "fp8c": lambda k, m, n, **kw: _build_fp8(k, m, n, m_block=512, n_chunk=1024,
                                             w_ld=16, **kw),
    "fp8d": lambda k, m, n, **kw: _build_fp8(k, m, n, m_block=512, n_chunk=1024,
                                             w_ld=32, w_bufs=2, merge_out=True,
                                             **kw),
    "fp8e": lambda k, m, n, **kw: _build_fp8(k, m, n, m_block=512, n_chunk=1024,
                                             w_ld=16, w_bufs=6, **kw),
    "fp8f": lambda k, m, n, **kw: _build_fp8(k, m, n, m_block=512, n_chunk=1024,
                                             w_ld=16, w_bufs=7, **kw),
    "fp8g": lambda k, m, n, **kw: _build_fp8(k, m, n, m_block=256, n_chunk=2048,
                                             w_ld=16, w_bufs=4, **kw),
    "fp8h": lambda k, m, n, **kw: _build_fp8(k, m, n, m_block=512, n_chunk=1024,
                                             w_ld=16, w_bufs=6, split_rings=True,
                                             **kw),
    "fp8_pe": lambda k, m, n, **kw: _build_fp8(k, m, n, m_block=512,
                                               n_chunk=1024, w_ld=16, w_bufs=6,
                                               pe_only=True, **kw),
    "fp8i": lambda k, m, n, **kw: _build_fp8(k, m, n, m_block=512, n_chunk=1024,
                                             w_ld=8, w_bufs=12, split_rings=True,
                                             **kw),
    "fp8j": lambda k, m, n, **kw: _build_fp8(k, m, n, m_block=512, n_chunk=1024,
                                             w_ld=8, w_bufs=12, split_rings=True,
                                             w_tiled=True, **kw),
    "bf1_nomm": _build_bf1_nomm,
    "bf1_nodma": _build_bf1_nodma,
}

# variant -> (m_block for host x layout, operand dtype, W pre-scale,
#             out is partition-major [P, m_core//P, n])
VARIANT_CFG = {
    "bf1": (512, BF16, 1.0, False),
    "fp8": (256, E4M3, W_SCALE, False),
    "fp8b": (512, E4M3, W_SCALE, False),
    "fp8c": (512, E4M3, W_SCALE, False),
    "fp8d": (512, E4M3, W_SCALE, True),
    "fp8e": (512, E4M3, W_SCALE, False),
    "fp8f": (512, E4M3, W_SCALE, False),
    "fp8g": (256, E4M3, W_SCALE, False),
    "fp8h": (512, E4M3, W_SCALE, False),
    "fp8i": (512, E4M3, W_SCALE, False),
    "fp8j": (512, E4M3, W_SCALE, False),
    "fp8_pe": (512, E4M3, W_SCALE, False),
    "bf1_nomm": (512, BF16, 1.0, False),
    "bf1_nodma": (512, BF16, 1.0, False),
}


def _variant():
    # fp8i: e4m3 DoubleRow, W in [P, 8, 1024] tiles x12 bufs on the sync ring,
    # x/out DMAs on the scalar ring. PE-only floor is ~732us (179 ns/MM);
    # fp8i measures 0.85-1.0 ms vs fp8e's 0.97-1.06 ms.
    return os.environ.get("KERNEL_VARIANT", "fp8i")


def _get_nc(k, m_core, n, **kw):
    variant = _variant()
    key = (variant, k, m_core, n, tuple(sorted(kw.items())))
    if key not in _BUILD_CACHE:
        _BUILD_CACHE[key] = _BUILDERS[variant](k, m_core, n, **kw)
    return _BUILD_CACHE[key]


def _to_pkm_blocks(a, m_block, dtype):
    """[rows, k] fp32 -> contiguous [n_mb, P, ko_n, m_block] in `dtype`
    (k = ko*128 + p)."""
    rows, k = a.shape
    n_mb = rows // m_block
    ko_n = k // P
    a = a.astype(dtype)
    a = a.reshape(n_mb, m_block, ko_n, P).transpose(0, 3, 2, 1)
    return np.ascontiguousarray(a)


def _w_to_pkn(w, dtype, scale=1.0):
    """[n, k] fp32 -> contiguous [P, ko_n, n] in `dtype`."""
    n, k = w.shape
    ko_n = k // P
    if scale != 1.0:
        w = w * np.float32(scale)
    w = w.astype(dtype)
    w = w.reshape(n, ko_n, P).transpose(2, 1, 0)
    return np.ascontiguousarray(w)


def _make_runner(nc):
    """Build the sharded PJRT executor for `nc` across the 8 cores.

    Mirrors concourse.bass2jax.run_bass_via_pjrt, but returns a reusable
    closure so repeated calls share one jit cache and inputs can stay
    device-resident for benchmarking.
    """
    import jax
    import concourse.mybir as mybir
    from concourse import bass2jax
    from jax.experimental.shard_map import shard_map
    from jax.sharding import Mesh, NamedSharding, PartitionSpec

    bass2jax.install_neuronx_cc_hook()

    partition_name = nc.partition_id_tensor.name if nc.partition_id_tensor else None
    assert nc.dbg_addr is None

    in_names, out_names, out_avals = [], [], []
    for alloc in nc.m.functions[0].allocations:
        if not isinstance(alloc, mybir.MemoryLocationSet):
            continue
        name = alloc.memorylocations[0].name
        if alloc.kind == "ExternalInput":
            if name != partition_name:
                in_names.append(name)
        elif alloc.kind == "ExternalOutput":
            out_names.append(name)
            out_avals.append(
                jax.core.ShapedArray(tuple(alloc.tensor_shape), mybir.dt.np(alloc.dtype))
            )
    n_params = len(in_names)
    n_outs = len(out_avals)
    all_in_names = tuple(in_names) + tuple(out_names)
    if partition_name is not None:
        all_in_names = all_in_names + (partition_name,)
    donate = tuple(range(n_params, n_params + n_outs))

    def _body(*args):
        operands = list(args)
        if partition_name is not None:
            operands.append(bass2jax.partition_id_tensor())
        outs = bass2jax._bass_exec_p.bind(
            *operands,
            out_avals=tuple(out_avals),
            in_names=all_in_names,
            out_names=tuple(out_names),
            lowering_input_output_aliases=(),
            sim_require_finite=True,
            sim_require_nnan=True,
            nc=nc,
        )
        return tuple(outs)

    devices = jax.devices()[:N_CORES]
    assert len(devices) == N_CORES
    mesh = Mesh(np.asarray(devices), ("core",))
    spec = PartitionSpec("core")
    sharded = jax.jit(
        shard_map(
            _body,
            mesh=mesh,
            in_specs=(spec,) * (n_params + n_outs),
            out_specs=(spec,) * n_outs,
            check_rep=False,
        ),
        donate_argnums=donate,
        keep_unused=True,
    )
    sharding = NamedSharding(mesh, spec)
    return {
        "sharded": sharded,
        "sharding": sharding,
        "in_names": in_names,
        "out_names": out_names,
        "out_avals": out_avals,
        "n_params": n_params,
        "n_outs": n_outs,
    }


def _get_runner(nc):
    key = id(nc)
    if key not in _RUNNER_CACHE:
        _RUNNER_CACHE[key] = _make_runner(nc)
    return _RUNNER_CACHE[key]


def _run(nc, in_maps):
    """Execute the kernel across 8 cores; returns per-core output dicts."""
    import numpy as np

    r = _get_runner(nc)
    n_cores = len(in_maps)
    concat_in = [
        np.concatenate([np.asarray(m[name]) for m in in_maps], axis=0)
        for name in r["in_names"]
    ]
    concat_zeros = [
        np.zeros((n_cores * a.shape[0], *a.shape[1:]), a.dtype) for a in r["out_avals"]
    ]
    out_arrs = r["sharded"](*concat_in, *concat_zeros)
    return [
        {
            name: np.asarray(out_arrs[i]).reshape(n_cores, *r["out_avals"][i].shape)[c]
            for i, name in enumerate(r["out_names"])
        }
        for c in range(n_cores)
    ]


def _bench(in_maps, k, m_core, n, reps):
    """Measure steady-state per-GEMM time: the kernel repeated `reps` times
    inside one program, minus the reps=1 program, divided by reps-1. Fixed
    dispatch overhead cancels in the difference. Sets LAST_EXEC_NS."""
    global LAST_EXEC_NS
    import time

    import jax
    import jax.numpy as jnp
    import numpy as np

    runners = {}
    dev_in = None
    for r_reps in (1, reps):
        nc = _get_nc(k, m_core, n, reps=r_reps)
        r = _get_runner(nc)
        runners[r_reps] = r
        if dev_in is None:
            concat_in = [
                np.concatenate([np.asarray(m[name]) for m in in_maps], axis=0)
                for name in r["in_names"]
            ]
            dev_in = [jax.device_put(a, r["sharding"]) for a in concat_in]
            jax.block_until_ready(dev_in)

    def _zeros(r):
        zs = [
            jax.jit(lambda a=a: jnp.zeros(a.shape, a.dtype),
                    out_shardings=r["sharding"])()
            for a in r["out_avals"]
        ]
        jax.block_until_ready(zs)
        return zs

    n_calls = int(os.environ.get("KERNEL_BENCH_CALLS", "3"))

    def _attempt(r_reps):
        # Time n_calls back-to-back dispatches with a single final sync:
        # per-call host/tunnel jitter amortizes across the batch.
        r = runners[r_reps]
        zsets = [_zeros(r) for _ in range(n_calls)]
        t0 = time.perf_counter()
        outs = [r["sharded"](*dev_in, *zs) for zs in zsets]
        jax.block_until_ready(outs)
        return (time.perf_counter() - t0) / n_calls

    for r_reps in (1, reps):  # compile + warmup both programs first
        _attempt(r_reps)

    # Interleaved attempt pairs: slow drift in fixed overhead is common-mode
    # within a pair, so per-pair deltas are far more stable than min-of-each.
    deltas, t1s, tns = [], [], []
    for _ in range(int(os.environ.get("KERNEL_BENCH_TRIES", "6"))):
        t1 = _attempt(1)
        tn = _attempt(reps)
        t1s.append(t1)
        tns.append(tn)
        deltas.append((tn - t1) / (reps - 1))
    per_iter = min(deltas)
    LAST_EXEC_NS = int(per_iter * 1e9)
    print(f"[bench] reps=1: {[f'{a * 1e3:.2f}' for a in t1s]}")
    print(f"[bench] reps={reps}: {[f'{a * 1e3:.2f}' for a in tns]}")
    print(f"[bench] per-GEMM deltas (ms): {[f'{d * 1e3:.3f}' for d in deltas]}")
    print(f"[bench] per-GEMM: {per_iter * 1e3:.3f} ms "
          f"(fixed+1iter: {min(t1s) * 1e3:.3f} ms)")


def kernel(input_, weight, bias):
    global LAST_RESULTS

    input_ = np.asarray(input_, dtype=np.float32)
    weight = np.asarray(weight, dtype=np.float32)
    bias = np.asarray(bias, dtype=np.float32)

    seq, batch, k = input_.shape
    n = weight.shape[0]
    m_full = seq * batch
    m_core = m_full // N_CORES

    variant = _variant()
    nc = _get_nc(k, m_core, n)

    x2 = input_.reshape(m_full, k)
    m_block, dtype, w_scale, out_pm = VARIANT_CFG[variant]
    wp = _w_to_pkn(weight, dtype, scale=w_scale)
    if variant == "fp8j":
        # repack [P, ko, n] -> [n_nc, kw, P, w_ld, n_chunk] tile-contiguous
        w_ld, n_chunk = 8, 1024
        ko_n = k // P
        wp = np.ascontiguousarray(
            wp.reshape(P, ko_n // w_ld, w_ld, n // n_chunk, n_chunk)
              .transpose(3, 1, 0, 2, 4)).reshape(-1, P, w_ld, n_chunk)

    in_maps = []
    for c in range(N_CORES):
        xp = _to_pkm_blocks(x2[c * m_core:(c + 1) * m_core], m_block, dtype)
        in_maps.append({"xb": xp, "wb": wp})

    results = _run(nc, in_maps)
    LAST_RESULTS = results

    bench_reps = int(os.environ.get("KERNEL_BENCH", "0"))
    if bench_reps > 1:
        _bench(in_maps, k, m_core, n, bench_reps)

    per_core = [results[c]["out"] for c in range(N_CORES)]
    if out_pm:
        # device layout [P, m_core//P, n]: row mg*P + p lives at [p, mg, :]
        per_core = [o.transpose(1, 0, 2).reshape(m_core, n) for o in per_core]
    out = np.concatenate(per_core, axis=0)
    if w_scale != 1.0:
        out = out * np.float32(1.0 / w_scale)
    out = out.reshape(seq, batch, n)
    if bias.any():
        out = out + bias
    return out
